# revision 2
# baseline (speedup 1.0000x reference)
"""GQA causal attention (B=1, S=4096, D=1024, H=16, HKV=4, Dh=64, RoPE) on
8 Trainium2 NeuronCores — v2 (software-pipelined single fused pass).

Sharding: 8-way head parallelism as v1 (core c owns query heads {2c, 2c+1},
sharing KV head c//2; host sums the 8 partial output projections in f64).

Device program (4 q-tiles of 1024, one TileContext):
  - Per tile t: scores S^T[k, q] per 128-key chunk (fp32r, two 512-wide
    matmuls into a double-buffered [128,1024] PSUM pair), exp on ScalarE
    (PSUM in, bf16 out, fixed bias -10 — softmax-shift-invariant), diagonal
    triangle zeroed by gpsimd affine_select. exp is the metronome: ScalarE
    runs one 0.9-1us exp per (head, chunk) and everything else is scheduled
    around keeping it saturated.
  - PV flipped: out[q(128), dh+1] accumulated per (q-chunk, k-chunk), e
    stationary, V[k, dh|ones] bf16 as 65-row moving operand (65 rows/matmul
    vs q-width in the natural orientation). Ones column -> per-partition
    softmax denominator, so normalize is reciprocal + tensor_scalar. The 8
    q-chunk accumulators live in 2 PSUM banks as interleaved accumulation
    groups (single bank-clearing start, per-element pending-zero handles
    first-write-overwrite). PV is deferred one chunk so it never blocks the
    next chunk's scores in the in-order PE queue.
  - Normalized O[q, hd] (f16) is PE-transposed to O^T per q-chunk for the
    output projection (f16 weights), staged f16, DMA'd per q-chunk row.
  - Cross-tile software pipeline: projections for tile t+1 (Q/K via
    w-stationary streams + rope; V via dh-stationary stream + PE transpose)
    and the previous tiles' output projections are split into ~0.2-0.7us
    micro-ops drained between chunk emissions, so no insertion head-blocks
    the in-order engine queues. Output projections are scheduled into the
    LATER tiles (t0->t2, t1,t2->t3) where ScalarE is the local bottleneck
    and PE has slack. The last tile's tail is normed per-q-chunk the moment
    its accumulator completes so the output tail overlaps the final chunks.
PSUM: s0,s1 (2 banks each) + oaccA,oaccB (1+1) + 2 rotating "op" banks = 8.
"""

import os

import numpy as np

B, S, D = 1, 4096, 1024
H, HKV, DH = 16, 4, 64
NCORES = 8
ROPE_THETA = 10000.0
QT = 1024
NQT = S // QT
EXP_BIAS = -10.0

_cache = {}


def _build_fast():
    import concourse.bass as bass
    import concourse.tile as tile
    from concourse import bacc, mybir
    from concourse.masks import make_identity

    f32 = mybir.dt.float32
    f32r = mybir.dt.float32r
    bf16 = mybir.dt.bfloat16
    f16 = mybir.dt.float16

    nc = bacc.Bacc(None, target_bir_lowering=False)

    xT = nc.dram_tensor("xT", [D, S], f32r, kind="ExternalInput")
    wqT = nc.dram_tensor("wqT", [D, 128], f32r, kind="ExternalInput")
    wkTd = nc.dram_tensor("wkTd", [D, 128], f32r, kind="ExternalInput")
    wvT = nc.dram_tensor("wvT", [D, DH], f32r, kind="ExternalInput")
    woT = nc.dram_tensor("woT", [128, D], f16, kind="ExternalInput")
    cosT = nc.dram_tensor("cosT", [128, S], f32, kind="ExternalInput")
    sinTs = nc.dram_tensor("sinTs", [128, S], f32, kind="ExternalInput")
    out = nc.dram_tensor("out", [S, D], f16, kind="ExternalOutput")

    with tile.TileContext(nc) as tc:
        with tc.tile_pool(name="const", bufs=1) as cpool, \
             tc.tile_pool(name="xb", bufs=2) as xb_pool, \
             tc.tile_pool(name="rtmp", bufs=2) as rtmp, \
             tc.tile_pool(name="esb", bufs=7) as e_pool, \
             tc.tile_pool(name="onp", bufs=2) as on_pool, \
             tc.tile_pool(name="ontp", bufs=4) as ont_pool, \
             tc.tile_pool(name="vts", bufs=2) as vt_pool, \
             tc.tile_pool(name="rcp", bufs=2) as rc_pool, \
             tc.tile_pool(name="obp", bufs=3) as ob_pool, \
             tc.tile_pool(name="psA", bufs=1, space="PSUM") as psA, \
             tc.tile_pool(name="psB", bufs=2, space="PSUM") as psB:

            # ---- resident constants ----
            wq_sb = cpool.tile([128, 8, 128], f32r)
            wk_sb = cpool.tile([128, 8, 128], f32r)
            wv_sb = cpool.tile([128, 8, DH], f32r)
            wo_sb = cpool.tile([128, D], f16)
            cos_sb = cpool.tile([128, S], f32)
            sin_sb = cpool.tile([128, S], f32)
            QTr = cpool.tile([128, S], f32r)   # rope(Q)^T rows 0-63 h0, 64-127 h1
            KTr = cpool.tile([128, S], f32r)   # rope(K)^T duplicated
            Vp = cpool.tile([128, S // 128, DH + 1], bf16)  # V[k, dh] + ones
            identb = cpool.tile([DH, DH], bf16)
            identh = cpool.tile([128, 128], f16)
            biasc = cpool.tile([128, 1], f32)

            nc.sync.dma_start(
                out=wq_sb, in_=wqT[:, :].rearrange("(c p) m -> p c m", p=128))
            nc.sync.dma_start(
                out=wk_sb, in_=wkTd[:, :].rearrange("(c p) m -> p c m", p=128))
            make_identity(nc, identb[:, :])
            make_identity(nc, identh[:, :])
            nc.vector.memset(biasc, float(EXP_BIAS))
            nc.vector.memset(Vp[:, :, DH:DH + 1], 1.0)

            xb_tiles = {}
            ON_t = {}
            ONT_t = {}
            SHUF = [i ^ 1 for i in range(32)]

            def emit_xb_dma(t, half):
                xb = xb_pool.tile([128, 4, QT], f32r, tag=f"xb{half}",
                                  name=f"xb_{t}_{half}")
                xb_tiles[(t, half)] = xb
                src = xT[half * 512:(half + 1) * 512, t * QT:(t + 1) * QT]
                nc.sync.dma_start(out=xb,
                                  in_=src.rearrange("(c p) q -> p c q", p=128))

            def emit_cs_dma(t, eng=None):
                # sync queue: the ScalarE sequencer must stay DMA-free so it
                # can dispatch exps (DMA issue blocks the issuing SEQ on the
                # serialized HWDGE)
                eng = eng or nc.sync
                sl = bass.ds(t * QT, QT)
                eng.dma_start(out=cos_sb[:, sl],
                              in_=cosT[:, t * QT:(t + 1) * QT])
                eng.dma_start(out=sin_sb[:, sl],
                              in_=sinTs[:, t * QT:(t + 1) * QT])

            def micro_qk(t, sgh, which, pool, tag):
                """Q/K projection stream + rope as a list of micro-ops."""
                sg = 2 * t + sgh
                st = {}
                w_sb = wq_sb if which == "q" else wk_sb
                dst = QTr if which == "q" else KTr
                scols = bass.ds(sg * 512, 512)

                def mk_mm(cd):
                    def f():
                        if "ps" not in st:
                            st["ps"] = pool.tile([128, 512], f32, tag=tag,
                                                 name=f"{which}t_{sg}")
                        nc.tensor.matmul(
                            st["ps"][:, :], w_sb[:, cd, :],
                            xb_tiles[(t, cd // 4)][:, cd % 4,
                                                   sgh * 512:(sgh + 1) * 512],
                            start=(cd == 0), stop=(cd == 7))
                    return f

                def rope_a():
                    st["m1"] = rtmp.tile([128, 512], f32, tag="m1", name=f"m1_{which}_{sg}")
                    st["m2"] = rtmp.tile([128, 512], f32, tag="m2", name=f"m2_{which}_{sg}")
                    nc.vector.tensor_mul(st["m1"], st["ps"][:, :],
                                         cos_sb[:, scols])
                    nc.vector.tensor_mul(st["m2"], st["ps"][:, :],
                                         sin_sb[:, scols])

                def rope_b():
                    sh = rtmp.tile([128, 512], f32, tag="sh")
                    nc.vector.stream_shuffle(sh, st["m2"], SHUF)
                    nc.vector.tensor_add(dst[:, scols], st["m1"], sh)

                return [mk_mm(cd) for cd in range(8)] + [rope_a, rope_b]

            def micro_vt(t, sgh):
                """V projection + transpose into Vp, as micro-ops."""
                sg = 2 * t + sgh
                st = {}

                def mk_mm(cd):
                    def f():
                        if "ps" not in st:
                            st["ps"] = psB.tile([DH, 512], f32, tag="op",
                                                name=f"vt_{sg}")
                        nc.tensor.matmul(
                            st["ps"][:, :], wv_sb[:, cd, :],
                            xb_tiles[(t, cd // 4)][:, cd % 4,
                                                   sgh * 512:(sgh + 1) * 512],
                            start=(cd == 0), stop=(cd == 7))
                    return f

                def cp():
                    st["vs"] = vt_pool.tile([DH, 512], bf16, tag="vt", name=f"vs_{sg}")
                    nc.vector.tensor_copy(st["vs"], st["ps"][:, :])

                def mk_tr(i):
                    def f():
                        kc = sg * 4 + i
                        tr = psB.tile([128, DH], bf16, tag="op",
                                      name=f"vtr_{kc}")
                        nc.tensor.transpose(
                            tr[:, :], st["vs"][:, i * 128:(i + 1) * 128],
                            identb[:, :])
                        nc.vector.tensor_copy(Vp[:, kc, 0:DH], tr[:, :])
                    return f

                return ([mk_mm(cd) for cd in range(8)] + [cp]
                        + [mk_tr(i) for i in range(4)])

            def emit_scores_exp(t, h, c, seg=None, e=None):
                """Scores + exp for key-chunk c over q-columns [lo, hi) of
                the tile (default: the full causal suffix)."""
                q0 = t * QT
                qs = max(0, (c - 8 * t) * 128)
                lo0, hi0 = (qs, QT) if seg is None else seg
                s_ps = psA.tile([128, QT], f32, tag=f"s{c % 2}",
                                name=f"s_{t}_{h}_{c}_{lo0}")
                lhs = KTr[64 * h:64 * h + 64, c * 128:(c + 1) * 128]
                for lo, hi in ((lo0, min(hi0, 512)), (max(lo0, 512), hi0)):
                    if lo >= hi:
                        continue
                    nc.tensor.matmul(
                        s_ps[:, bass.ds(lo, hi - lo)], lhs,
                        QTr[64 * h:64 * h + 64, q0 + lo:q0 + hi],
                        start=True, stop=True)
                if e is None:
                    e = e_pool.tile([128, QT], bf16, tag="e",
                                    name=f"e_{t}_{h}_{c}")
                nc.scalar.activation(
                    e[:, lo0:hi0], s_ps[:, lo0:hi0],
                    mybir.ActivationFunctionType.Exp,
                    bias=biasc[:, :], scale=1.0)
                if c >= 8 * t and lo0 <= qs < hi0:
                    nc.gpsimd.affine_select(
                        out=e[:, qs:qs + 128], in_=e[:, qs:qs + 128],
                        pattern=[[1, 128]],
                        compare_op=mybir.AluOpType.is_ge,
                        fill=0.0, base=0, channel_multiplier=-1)
                return e

            def emit_pv(t, c, e, bankA, bankB):
                for j in range(max(0, c - 8 * t), 8):
                    bank, jj = (bankA, j) if j < 4 else (bankB, j - 4)
                    last_c = 8 * t + (3 if j < 4 else 7)
                    nc.tensor.matmul(
                        bank[:, jj, :], e[:, j * 128:(j + 1) * 128],
                        Vp[:, c, :],
                        start=(c == 0 and jj == 0),
                        stop=(c == last_c and jj == 3))

            def emit_norm(t, h, bank, jbase, js):
                rc = rc_pool.tile([128, 4, 1], f32, tag="rc",
                                  name=f"rc_{t}_{h}_{jbase}_{js[0]}")
                j0, j1 = js[0], js[-1] + 1
                nc.vector.reciprocal_approx_fast(
                    rc[:, 0:j1 - j0, :], bank[:, j0:j1, DH:DH + 1])
                for j4 in js:
                    nc.vector.tensor_scalar_mul(
                        ON_t[t][:, jbase + j4, 64 * h:64 * h + 64],
                        bank[:, j4, 0:DH], rc[:, j4 - j0, :])

            def emit_ont(t, j):
                tr = psB.tile([128, 128], f16, tag="op", name=f"ontr_{t}_{j}")
                nc.tensor.transpose(tr[:, :], ON_t[t][:, j, :], identh[:, :])
                nc.vector.tensor_copy(ONT_t[t][:, j, :], tr[:, :])

            def micro_oproj(t, j, split_eng=False):
                """Output projection for q-chunk j of tile t: two half-units.
                split_eng puts the first staging copy on ScalarE (tail mode,
                when ScalarE has gone idle)."""
                st = {}

                def half(dseg):
                    def f():
                        if "ob" not in st:
                            st["ob"] = ob_pool.tile([128, QT], f16, tag="ob", name=f"ob_{t}_{j}")
                        op = psB.tile([128, 512], f32, tag="op",
                                      name=f"op_{t}_{j}_{dseg}")
                        nc.tensor.matmul(
                            op[:, :], ONT_t[t][:, j, :],
                            wo_sb[:, dseg * 512:(dseg + 1) * 512],
                            start=True, stop=True)
                        eng = nc.scalar if (split_eng and dseg == 0) \
                            else nc.vector
                        if eng is nc.scalar:
                            eng.copy(st["ob"][:, dseg * 512:(dseg + 1) * 512],
                                     op[:, :])
                        else:
                            eng.tensor_copy(
                                st["ob"][:, dseg * 512:(dseg + 1) * 512],
                                op[:, :])
                        if dseg == 1:
                            nc.sync.dma_start(
                                out=out[t * QT + j * 128:
                                        t * QT + (j + 1) * 128, :],
                                in_=st["ob"])
                    return f

                return [half(0), half(1)]

            # ---- prologue: only what chunk 0-3's first 512 q-columns need
            # (Q/K of column group 0); the rest is emitted at the phase
            # boundary inside tile 0 so the first exps start ~15us earlier.
            # DMA issue order is deliberate: small weight/table DMAs first,
            # then the big x transfers, all on the sync queue.
            emit_cs_dma(0)
            emit_xb_dma(0, 0)
            emit_xb_dma(0, 1)
            nc.sync.dma_start(
                out=wv_sb, in_=wvT[:, :].rearrange("(c p) m -> p c m", p=128))
            for f in micro_qk(0, 0, "q", psA, "s0"):
                f()
            for f in micro_qk(0, 0, "k", psA, "s1"):
                f()

            pend = {"pv": None, "tail": None}
            for t in range(NQT):
                NCH = 8 * (t + 1)
                last_t = t == NQT - 1
                ON_t[t] = on_pool.tile([128, 8, 128], f16, tag="ON",
                                       name=f"ON_{t}")
                if last_t:
                    ONT_t[t] = ont_pool.tile([128, 8, 128], f16, tag="ONT",
                                             name=f"ONT_{t}")
                # next tile's input DMAs first (latency-critical)
                if t + 1 < NQT:
                    emit_xb_dma(t + 1, 0)
                    emit_xb_dma(t + 1, 1)
                    emit_cs_dma(t + 1)
                if t == 0:
                    nc.sync.dma_start(out=wo_sb, in_=woT[:, :])

                # early queue: ONT transposes of t-1 + scheduled oproj units
                early = []
                if t >= 1:
                    ONT_t[t - 1] = ont_pool.tile([128, 8, 128], f16,
                                                 tag="ONT",
                                                 name=f"ONT_{t-1}")
                    for j in range(8):
                        early.append(lambda t=t, j=j: emit_ont(t - 1, j))
                # oproj schedule: t0 -> tile2, t1 and t2 -> tile3
                osrc = {2: [0], 3: [1, 2]}.get(t, [])
                for ot in osrc:
                    for j in range(8):
                        early.extend(micro_oproj(ot, j))
                # late queue: projections for tile t+1 (needs xb DMA landed)
                late = []
                if t + 1 < NQT:
                    order = ([(0, "q"), (0, "k"), (1, "q"), (1, "k")]
                             if t == 0 else
                             [(0, "q"), (0, "k"), (1, "q"), (1, "k")])
                    for sgh, which in order:
                        late.extend(micro_qk(t + 1, sgh, which, psB, "op"))
                    late.extend(micro_vt(t + 1, 0))
                    late.extend(micro_vt(t + 1, 1))

                iters = 2 * NCH
                n_early = len(early)
                n_late = len(late)
                done_iters = 0
                e_popped = l_popped = 0
                LATE_FRAC = 0.30 if t == 0 else 0.35
                for h in range(2):
                    es0 = {}
                    if t == 0 and h == 0:
                        # phase A: first 512 q-columns of chunks 0-3 need
                        # only column-group-0 Q/K (already roped) — start
                        # ScalarE while the rest of the projections build
                        for c in range(4):
                            es0[c] = emit_scores_exp(0, 0, c,
                                                     seg=(128 * c, 512))
                        # phase boundary: column-group-1 Q/K + V projections
                        for f in micro_qk(0, 1, "q", psB, "op"):
                            f()
                        for f in micro_qk(0, 1, "k", psB, "op"):
                            f()
                        for f in micro_vt(0, 0) + micro_vt(0, 1):
                            f()
                    bankA = psA.tile([128, 4, DH + 1], f32, tag="oaccA",
                                     name=f"oA_{t}_{h}")
                    bankB = psA.tile([128, 4, DH + 1], f32, tag="oaccB",
                                     name=f"oB_{t}_{h}")
                    for c in range(NCH):
                        if c in es0:
                            e = emit_scores_exp(t, h, c, seg=(512, QT),
                                                e=es0[c])
                        else:
                            e = emit_scores_exp(t, h, c)
                        if pend["pv"] is not None:
                            emit_pv(*pend["pv"])
                            pend["pv"] = None
                        if pend["tail"] is not None:
                            pend["tail"]()
                            pend["tail"] = None
                        pend["pv"] = (t, c, e, bankA, bankB)
                        cj = c - 8 * t
                        if cj == 4:
                            emit_norm(t, h, bankA, 0, (0, 1, 2, 3))
                            if last_t and h == 1:
                                for j in range(4):
                                    emit_ont(t, j)
                                    for f in micro_oproj(t, j,
                                                         split_eng=True):
                                        f()
                        if last_t and cj >= 5:
                            jd = cj - 1
                            emit_norm(t, h, bankB, 4, (jd - 4,))
                            if h == 1:
                                emit_ont(t, jd)
                                for f in micro_oproj(t, jd, split_eng=True):
                                    f()
                        done_iters += 1
                        et = (n_early * done_iters * 4 + 3 * iters) \
                            // (3 * iters)
                        while e_popped < min(et, n_early):
                            early[e_popped]()
                            e_popped += 1
                        prog = done_iters / iters
                        if prog > LATE_FRAC:
                            lt = int(n_late * (prog - LATE_FRAC)
                                     / (0.95 - LATE_FRAC)) + 1
                            while l_popped < min(lt, n_late):
                                late[l_popped]()
                                l_popped += 1
                    # defer this head's final PV + bank-B norm past the next
                    # head's/tile's first scores+exp (no PE head-block)
                    if not (last_t and h == 1):
                        def _tail(t=t, h=h, bankB=bankB, pv=pend["pv"],
                                  lt=last_t):
                            emit_pv(*pv)
                            emit_norm(t, h, bankB, 4, (3,) if lt else
                                      (0, 1, 2, 3))
                        pend["pv"] = None
                        pend["tail"] = _tail
                    else:
                        emit_pv(*pend["pv"])
                        pend["pv"] = None
                        emit_norm(t, h, bankB, 4, (3,))
                        emit_ont(t, 7)
                        for f in micro_oproj(t, 7, split_eng=True):
                            f()
                while e_popped < n_early:
                    early[e_popped]()
                    e_popped += 1
                while l_popped < n_late:
                    late[l_popped]()
                    l_popped += 1

    nc.compile()
    return nc


def _host_inputs(x, wq, wk, wv, wo):
    """Build the 8 per-core input dicts."""
    x2 = np.ascontiguousarray(x.reshape(S, D))
    xT = np.ascontiguousarray(x2.T)

    # rope pair-interleaved dh order: [0, 32, 1, 33, ...]
    perm = np.empty(DH, dtype=np.int64)
    perm[0::2] = np.arange(DH // 2)
    perm[1::2] = np.arange(DH // 2) + DH // 2

    inv_freq = 1.0 / (ROPE_THETA ** (np.arange(0, DH, 2, dtype=np.float64) / DH))
    ang = np.arange(S, dtype=np.float64)[:, None] * inv_freq[None, :]  # [S, 32]
    cosv = np.cos(ang)
    sinv = np.sin(ang)
    C64 = np.empty((DH, S), dtype=np.float32)
    Ss64 = np.empty((DH, S), dtype=np.float32)
    for j in range(DH):
        C64[j] = cosv[:, j // 2]
        Ss64[j] = sinv[:, j // 2] * (1.0 if j % 2 == 0 else -1.0)
    cosT = np.ascontiguousarray(np.tile(C64, (2, 1)))
    sinTs = np.ascontiguousarray(np.tile(Ss64, (2, 1)))

    wq4 = wq.reshape(H, DH, D)
    wk4 = wk.reshape(HKV, DH, D)
    wv4 = wv.reshape(HKV, DH, D)

    ins = []
    for c in range(NCORES):
        h0, h1 = 2 * c, 2 * c + 1
        g = h0 // (H // HKV)
        wq_c = np.concatenate([wq4[h0][perm], wq4[h1][perm]], axis=0)  # [128, D]
        wk_c = np.concatenate([wk4[g][perm], wk4[g][perm]], axis=0)    # [128, D]
        wo_c = wo[:, np.r_[h0 * DH:(h0 + 1) * DH, h1 * DH:(h1 + 1) * DH]]
        ins.append({
            "xT": xT,
            "wqT": np.ascontiguousarray(wq_c.T),
            "wkTd": np.ascontiguousarray(wk_c.T),
            "wvT": np.ascontiguousarray(wv4[g].T),
            "woT": np.ascontiguousarray(wo_c.T).astype(np.float16),
            "cosT": cosT,
            "sinTs": sinTs,
        })
    return ins


def _is_causal(mask):
    if mask.shape != (S, S):
        return False
    expected = np.where(np.tril(np.ones((S, S), dtype=bool)), np.float32(0.0),
                        np.float32(-1e9))
    return np.array_equal(mask, expected)


def run_cores(x, mask, wq, wk, wv, wo, **spmd_kwargs):
    from concourse.bass_utils import run_bass_kernel_spmd

    causal = _is_causal(np.asarray(mask))
    assert causal, "v2 fast path requires the standard causal mask"
    if True not in _cache:
        _cache[True] = _build_fast()
    nc = _cache[True]

    ins = _host_inputs(np.asarray(x), np.asarray(wq), np.asarray(wk),
                       np.asarray(wv), np.asarray(wo))
    res = run_bass_kernel_spmd(nc, ins, core_ids=list(range(NCORES)),
                               **spmd_kwargs)
    return res


def _build(causal):
    assert causal
    return _build_fast()


def kernel(x, mask, wq, wk, wv, wo):
    res = run_cores(x, mask, wq, wk, wv, wo)
    acc = np.zeros((S, D), dtype=np.float64)
    for r in res.results:
        acc += r["out"].astype(np.float64)
    return acc.astype(np.float32).reshape(B, S, D)


# revision 3
# speedup vs baseline: 1.0202x; 1.0202x over previous
"""GQA causal attention (B=1, S=4096, D=1024, H=16, HKV=4, Dh=64, RoPE) on
8 Trainium2 NeuronCores — v2 (software-pipelined single fused pass).

Sharding: 8-way head parallelism as v1 (core c owns query heads {2c, 2c+1},
sharing KV head c//2; host sums the 8 partial output projections in f64).

Device program (4 q-tiles of 1024, one TileContext):
  - Per tile t: scores S^T[k, q] per 128-key chunk (fp32r, two 512-wide
    matmuls into a double-buffered [128,1024] PSUM pair), exp on ScalarE
    (PSUM in, bf16 out, fixed bias -10 — softmax-shift-invariant), diagonal
    triangle zeroed by gpsimd affine_select. exp is the metronome: ScalarE
    runs one 0.9-1us exp per (head, chunk) and everything else is scheduled
    around keeping it saturated.
  - PV flipped: out[q(128), dh+1] accumulated per (q-chunk, k-chunk), e
    stationary, V[k, dh|ones] bf16 as 65-row moving operand (65 rows/matmul
    vs q-width in the natural orientation). Ones column -> per-partition
    softmax denominator, so normalize is reciprocal + tensor_scalar. The 8
    q-chunk accumulators live in 2 PSUM banks as interleaved accumulation
    groups (single bank-clearing start, per-element pending-zero handles
    first-write-overwrite). PV is deferred one chunk so it never blocks the
    next chunk's scores in the in-order PE queue.
  - Normalized O[q, hd] (f16) is PE-transposed to O^T per q-chunk for the
    output projection (f16 weights), staged f16, DMA'd per q-chunk row.
  - Cross-tile software pipeline: projections for tile t+1 (Q/K
    w-stationary streams + rope on DVE; V + PE transpose into Vp) and the
    previous tiles' output projections are split into ~0.2-0.7us micro-ops
    drained between chunk emissions, so no insertion head-blocks the
    in-order engine queues. All input DMAs ride the sync queue (a DMA issue
    blocks the issuing sequencer on the serialized HWDGE, so ScalarE's
    queue stays DMA-free to dispatch exps). Output projections are scheduled into the
    LATER tiles (t0->t2, t1,t2->t3) where ScalarE is the local bottleneck
    and PE has slack. The last tile's tail is normed per-q-chunk the moment
    its accumulator completes so the output tail overlaps the final chunks.
PSUM: s0,s1 (2 banks each) + oaccA,oaccB (1+1) + 2 rotating "op" banks = 8.
"""

import os

import numpy as np

B, S, D = 1, 4096, 1024
H, HKV, DH = 16, 4, 64
NCORES = 8
ROPE_THETA = 10000.0
QT = 1024
NQT = S // QT
EXP_BIAS = -10.0

_cache = {}


def _build_fast():
    import concourse.bass as bass
    import concourse.tile as tile
    from concourse import bacc, mybir
    from concourse.masks import make_identity

    f32 = mybir.dt.float32
    f32r = mybir.dt.float32r
    bf16 = mybir.dt.bfloat16
    f16 = mybir.dt.float16

    nc = bacc.Bacc(None, target_bir_lowering=False)

    xT = nc.dram_tensor("xT", [D, S], f32r, kind="ExternalInput")
    wqT = nc.dram_tensor("wqT", [D, 128], f32r, kind="ExternalInput")
    wkTd = nc.dram_tensor("wkTd", [D, 128], f32r, kind="ExternalInput")
    wvT = nc.dram_tensor("wvT", [D, DH], f32r, kind="ExternalInput")
    woT = nc.dram_tensor("woT", [128, D], f16, kind="ExternalInput")
    cosT = nc.dram_tensor("cosT", [128, S], f32, kind="ExternalInput")
    sinTs = nc.dram_tensor("sinTs", [128, S], f32, kind="ExternalInput")
    out = nc.dram_tensor("out", [S, D], f16, kind="ExternalOutput")

    with tile.TileContext(nc) as tc:
        with tc.tile_pool(name="const", bufs=1) as cpool, \
             tc.tile_pool(name="xb", bufs=2) as xb_pool, \
             tc.tile_pool(name="rtmp", bufs=2) as rtmp, \
             tc.tile_pool(name="esb", bufs=7) as e_pool, \
             tc.tile_pool(name="onp", bufs=2) as on_pool, \
             tc.tile_pool(name="ontp", bufs=4) as ont_pool, \
             tc.tile_pool(name="vts", bufs=2) as vt_pool, \
             tc.tile_pool(name="rcp", bufs=2) as rc_pool, \
             tc.tile_pool(name="txp", bufs=2) as tx_pool, \
             tc.tile_pool(name="obp", bufs=3) as ob_pool, \
             tc.tile_pool(name="psA", bufs=1, space="PSUM") as psA, \
             tc.tile_pool(name="psB", bufs=2, space="PSUM") as psB:

            # ---- resident constants ----
            wq_sb = cpool.tile([128, 8, 128], f32r)
            wk_sb = cpool.tile([128, 8, 128], f32r)
            wv_sb = cpool.tile([128, 8, DH], f32r)
            wo_sb = cpool.tile([128, D], f16)
            cos_sb = cpool.tile([128, S], f32)
            sin_sb = cpool.tile([128, S], f32)
            QTr = cpool.tile([128, S], f32r)   # rope(Q)^T rows 0-63 h0, 64-127 h1
            KTr = cpool.tile([128, S], f32r)   # rope(K)^T duplicated
            Vp = cpool.tile([128, S // 128, DH + 1], bf16)  # V[k, dh] + ones
            identb = cpool.tile([DH, DH], bf16)
            identh = cpool.tile([128, 128], f16)
            biasc = cpool.tile([128, 1], f32)

            nc.sync.dma_start(
                out=wq_sb, in_=wqT[:, :].rearrange("(c p) m -> p c m", p=128))
            nc.sync.dma_start(
                out=wk_sb, in_=wkTd[:, :].rearrange("(c p) m -> p c m", p=128))
            make_identity(nc, identb[:, :])
            make_identity(nc, identh[:, :])
            nc.vector.memset(biasc, float(EXP_BIAS))
            nc.vector.memset(Vp[:, :, DH:DH + 1], 1.0)

            xb_tiles = {}
            ON_t = {}
            ONT_t = {}
            SHUF = [i ^ 1 for i in range(32)]

            def emit_xb_dma(t, half):
                xb = xb_pool.tile([128, 4, QT], f32r, tag=f"xb{half}",
                                  name=f"xb_{t}_{half}")
                xb_tiles[(t, half)] = xb
                src = xT[half * 512:(half + 1) * 512, t * QT:(t + 1) * QT]
                nc.sync.dma_start(out=xb,
                                  in_=src.rearrange("(c p) q -> p c q", p=128))

            def emit_cs_dma(t, eng=None):
                # sync queue: the ScalarE sequencer must stay DMA-free so it
                # can dispatch exps (DMA issue blocks the issuing SEQ on the
                # serialized HWDGE)
                eng = eng or nc.sync
                sl = bass.ds(t * QT, QT)
                eng.dma_start(out=cos_sb[:, sl],
                              in_=cosT[:, t * QT:(t + 1) * QT])
                eng.dma_start(out=sin_sb[:, sl],
                              in_=sinTs[:, t * QT:(t + 1) * QT])

            def micro_qk(t, sgh, which, pool, tag):
                """Q/K projection stream + rope as a list of micro-ops."""
                sg = 2 * t + sgh
                st = {}
                w_sb = wq_sb if which == "q" else wk_sb
                dst = QTr if which == "q" else KTr
                scols = bass.ds(sg * 512, 512)

                def mk_mm(cd):
                    def f():
                        if "ps" not in st:
                            st["ps"] = pool.tile([128, 512], f32, tag=tag,
                                                 name=f"{which}t_{sg}")
                        nc.tensor.matmul(
                            st["ps"][:, :], w_sb[:, cd, :],
                            xb_tiles[(t, cd // 4)][:, cd % 4,
                                                   sgh * 512:(sgh + 1) * 512],
                            start=(cd == 0), stop=(cd == 7))
                    return f

                def rope_a():
                    st["m1"] = rtmp.tile([128, 512], f32, tag="m1", name=f"m1_{which}_{sg}")
                    st["m2"] = rtmp.tile([128, 512], f32, tag="m2", name=f"m2_{which}_{sg}")
                    nc.vector.tensor_mul(st["m1"], st["ps"][:, :],
                                         cos_sb[:, scols])
                    nc.vector.tensor_mul(st["m2"], st["ps"][:, :],
                                         sin_sb[:, scols])

                def rope_b():
                    sh = rtmp.tile([128, 512], f32, tag="sh")
                    nc.vector.stream_shuffle(sh, st["m2"], SHUF)
                    nc.vector.tensor_add(dst[:, scols], st["m1"], sh)

                return [mk_mm(cd) for cd in range(8)] + [rope_a, rope_b]

            def micro_vt(t, sgh):
                """V projection + transpose into Vp, as micro-ops."""
                sg = 2 * t + sgh
                st = {}

                def mk_mm(cd):
                    def f():
                        if "ps" not in st:
                            st["ps"] = psB.tile([DH, 512], f32, tag="op",
                                                name=f"vt_{sg}")
                        nc.tensor.matmul(
                            st["ps"][:, :], wv_sb[:, cd, :],
                            xb_tiles[(t, cd // 4)][:, cd % 4,
                                                   sgh * 512:(sgh + 1) * 512],
                            start=(cd == 0), stop=(cd == 7))
                    return f

                def cp():
                    st["vs"] = vt_pool.tile([DH, 512], bf16, tag="vt", name=f"vs_{sg}")
                    nc.vector.tensor_copy(st["vs"], st["ps"][:, :])

                def mk_tr(i):
                    def f():
                        kc = sg * 4 + i
                        tr = psB.tile([128, DH], bf16, tag="op",
                                      name=f"vtr_{kc}")
                        nc.tensor.transpose(
                            tr[:, :], st["vs"][:, i * 128:(i + 1) * 128],
                            identb[:, :])
                        nc.vector.tensor_copy(Vp[:, kc, 0:DH], tr[:, :])
                    return f

                return ([mk_mm(cd) for cd in range(8)] + [cp]
                        + [mk_tr(i) for i in range(4)])

            # Schraudolph-style integer exp producing bf16 directly:
            #   e = bitcast_bf16(uint16(max(A*s + B, 0)))   (~4% max rel err)
            # used to offload some exps from the saturated ScalarE onto
            # DVE (affine, PSUM read) + GpSimd (clamp + u16 convert).
            SCH_A = float(128.0 / np.log(2.0))
            SCH_B = float(127 * 128 - 4.0 + SCH_A * EXP_BIAS)
            u16 = mybir.dt.uint16

            def emit_scores_exp(t, h, c, seg=None, e=None, approx=False):
                """Scores + exp for key-chunk c over q-columns [lo, hi) of
                the tile (default: the full causal suffix)."""
                q0 = t * QT
                qs = max(0, (c - 8 * t) * 128)
                lo0, hi0 = (qs, QT) if seg is None else seg
                s_ps = psA.tile([128, QT], f32, tag=f"s{c % 2}",
                                name=f"s_{t}_{h}_{c}_{lo0}")
                lhs = KTr[64 * h:64 * h + 64, c * 128:(c + 1) * 128]
                for lo, hi in ((lo0, min(hi0, 512)), (max(lo0, 512), hi0)):
                    if lo >= hi:
                        continue
                    nc.tensor.matmul(
                        s_ps[:, bass.ds(lo, hi - lo)], lhs,
                        QTr[64 * h:64 * h + 64, q0 + lo:q0 + hi],
                        start=True, stop=True)
                if e is None:
                    e = e_pool.tile([128, QT], bf16, tag="e",
                                    name=f"e_{t}_{h}_{c}")
                if approx:
                    tx = tx_pool.tile([128, QT], f32, tag="tx",
                                      name=f"tx_{t}_{h}_{c}")
                    nc.vector.tensor_scalar(
                        tx[:, lo0:hi0], s_ps[:, lo0:hi0], SCH_A, SCH_B,
                        mybir.AluOpType.mult, mybir.AluOpType.add)
                    nc.gpsimd.tensor_scalar(
                        e[:, lo0:hi0].bitcast(u16), tx[:, lo0:hi0],
                        0.0, None, mybir.AluOpType.max)
                else:
                    nc.scalar.activation(
                        e[:, lo0:hi0], s_ps[:, lo0:hi0],
                        mybir.ActivationFunctionType.Exp,
                        bias=biasc[:, :], scale=1.0)
                if c >= 8 * t and lo0 <= qs < hi0:
                    nc.gpsimd.affine_select(
                        out=e[:, qs:qs + 128], in_=e[:, qs:qs + 128],
                        pattern=[[1, 128]],
                        compare_op=mybir.AluOpType.is_ge,
                        fill=0.0, base=0, channel_multiplier=-1)
                return e

            def emit_pv(t, c, e, bankA, bankB):
                for j in range(max(0, c - 8 * t), 8):
                    bank, jj = (bankA, j) if j < 4 else (bankB, j - 4)
                    last_c = 8 * t + (3 if j < 4 else 7)
                    nc.tensor.matmul(
                        bank[:, jj, :], e[:, j * 128:(j + 1) * 128],
                        Vp[:, c, :],
                        start=(c == 0 and jj == 0),
                        stop=(c == last_c and jj == 3))

            def emit_norm(t, h, bank, jbase, js):
                rc = rc_pool.tile([128, 4, 1], f32, tag="rc",
                                  name=f"rc_{t}_{h}_{jbase}_{js[0]}")
                j0, j1 = js[0], js[-1] + 1
                nc.vector.reciprocal_approx_fast(
                    rc[:, 0:j1 - j0, :], bank[:, j0:j1, DH:DH + 1])
                for j4 in js:
                    nc.vector.tensor_scalar_mul(
                        ON_t[t][:, jbase + j4, 64 * h:64 * h + 64],
                        bank[:, j4, 0:DH], rc[:, j4 - j0, :])

            def emit_ont(t, j):
                tr = psB.tile([128, 128], f16, tag="op", name=f"ontr_{t}_{j}")
                nc.tensor.transpose(tr[:, :], ON_t[t][:, j, :], identh[:, :])
                nc.vector.tensor_copy(ONT_t[t][:, j, :], tr[:, :])

            def micro_oproj(t, j, split_eng=False):
                """Output projection for q-chunk j of tile t: two half-units.
                split_eng puts the first staging copy on ScalarE (tail mode,
                when ScalarE has gone idle)."""
                st = {}

                def half(dseg):
                    def f():
                        if "ob" not in st:
                            st["ob"] = ob_pool.tile([128, QT], f16, tag="ob", name=f"ob_{t}_{j}")
                        op = psB.tile([128, 512], f32, tag="op",
                                      name=f"op_{t}_{j}_{dseg}")
                        nc.tensor.matmul(
                            op[:, :], ONT_t[t][:, j, :],
                            wo_sb[:, dseg * 512:(dseg + 1) * 512],
                            start=True, stop=True)
                        eng = nc.scalar if (split_eng and dseg == 0) \
                            else nc.vector
                        if eng is nc.scalar:
                            eng.copy(st["ob"][:, dseg * 512:(dseg + 1) * 512],
                                     op[:, :])
                        else:
                            eng.tensor_copy(
                                st["ob"][:, dseg * 512:(dseg + 1) * 512],
                                op[:, :])
                        if dseg == 1:
                            nc.sync.dma_start(
                                out=out[t * QT + j * 128:
                                        t * QT + (j + 1) * 128, :],
                                in_=st["ob"])
                    return f

                return [half(0), half(1)]

            # ---- prologue: only what chunk 0-3's first 512 q-columns need
            # (Q/K of column group 0); the rest is emitted at the phase
            # boundary inside tile 0 so the first exps start ~15us earlier.
            # DMA issue order is deliberate: small weight/table DMAs first,
            # then the big x transfers, all on the sync queue.
            emit_cs_dma(0)
            emit_xb_dma(0, 0)
            emit_xb_dma(0, 1)
            nc.sync.dma_start(
                out=wv_sb, in_=wvT[:, :].rearrange("(c p) m -> p c m", p=128))
            for f in micro_qk(0, 0, "q", psA, "s0"):
                f()
            for f in micro_qk(0, 0, "k", psA, "s1"):
                f()

            pend = {"pv": None, "tail": None}
            for t in range(NQT):
                NCH = 8 * (t + 1)
                last_t = t == NQT - 1
                ON_t[t] = on_pool.tile([128, 8, 128], f16, tag="ON",
                                       name=f"ON_{t}")
                if last_t:
                    ONT_t[t] = ont_pool.tile([128, 8, 128], f16, tag="ONT",
                                             name=f"ONT_{t}")
                # next tile's input DMAs first (latency-critical)
                if t + 1 < NQT:
                    emit_xb_dma(t + 1, 0)
                    emit_xb_dma(t + 1, 1)
                    emit_cs_dma(t + 1)
                if t == 0:
                    nc.sync.dma_start(out=wo_sb, in_=woT[:, :])

                # early queue: ONT transposes of t-1 + scheduled oproj units
                early = []
                if t >= 1:
                    ONT_t[t - 1] = ont_pool.tile([128, 8, 128], f16,
                                                 tag="ONT",
                                                 name=f"ONT_{t-1}")
                    for j in range(8):
                        early.append(lambda t=t, j=j: emit_ont(t - 1, j))
                # oproj schedule: t0 -> tile2, t1 and t2 -> tile3
                osrc = {2: [0], 3: [1, 2]}.get(t, [])
                for ot in osrc:
                    for j in range(8):
                        early.extend(micro_oproj(ot, j))
                # late queue: projections for tile t+1 (needs xb DMA landed)
                late = []
                if t + 1 < NQT:
                    order = ([(0, "q"), (0, "k"), (1, "q"), (1, "k")]
                             if t == 0 else
                             [(0, "q"), (0, "k"), (1, "q"), (1, "k")])
                    for sgh, which in order:
                        late.extend(micro_qk(t + 1, sgh, which, psB, "op"))
                    late.extend(micro_vt(t + 1, 0))
                    late.extend(micro_vt(t + 1, 1))

                iters = 2 * NCH
                n_early = len(early)
                n_late = len(late)
                done_iters = 0
                e_popped = l_popped = 0
                LATE_FRAC = 0.30 if t == 0 else 0.35
                for h in range(2):
                    es0 = {}
                    if t == 0 and h == 0:
                        # phase A: first 512 q-columns of chunks 0-3 need
                        # only column-group-0 Q/K (already roped) — start
                        # ScalarE while the rest of the projections build
                        for c in range(4):
                            es0[c] = emit_scores_exp(0, 0, c,
                                                     seg=(128 * c, 512))
                        # phase boundary: column-group-1 Q/K + V projections
                        for f in micro_qk(0, 1, "q", psB, "op"):
                            f()
                        for f in micro_qk(0, 1, "k", psB, "op"):
                            f()
                        for f in micro_vt(0, 0) + micro_vt(0, 1):
                            f()
                    bankA = psA.tile([128, 4, DH + 1], f32, tag="oaccA",
                                     name=f"oA_{t}_{h}")
                    bankB = psA.tile([128, 4, DH + 1], f32, tag="oaccB",
                                     name=f"oB_{t}_{h}")
                    for c in range(NCH):
                        off = False  # ScalarE->DVE/GpSimd exp offload: net loss (queue serialization)
                        if c in es0:
                            e = emit_scores_exp(t, h, c, seg=(512, QT),
                                                e=es0[c])
                        else:
                            e = emit_scores_exp(t, h, c, approx=off)
                        if pend["pv"] is not None:
                            emit_pv(*pend["pv"])
                            pend["pv"] = None
                        if pend["tail"] is not None:
                            pend["tail"]()
                            pend["tail"] = None
                        pend["pv"] = (t, c, e, bankA, bankB)
                        cj = c - 8 * t
                        if cj == 4:
                            emit_norm(t, h, bankA, 0, (0, 1, 2, 3))
                            if last_t and h == 1:
                                for j in range(4):
                                    emit_ont(t, j)
                                    for f in micro_oproj(t, j,
                                                         split_eng=True):
                                        f()
                        if last_t and cj >= 5:
                            jd = cj - 1
                            emit_norm(t, h, bankB, 4, (jd - 4,))
                            if h == 1:
                                emit_ont(t, jd)
                                for f in micro_oproj(t, jd, split_eng=True):
                                    f()
                        done_iters += 1
                        et = (n_early * done_iters * 4 + 3 * iters) \
                            // (3 * iters)
                        while e_popped < min(et, n_early):
                            early[e_popped]()
                            e_popped += 1
                        prog = done_iters / iters
                        if prog > LATE_FRAC:
                            lt = int(n_late * (prog - LATE_FRAC)
                                     / (0.95 - LATE_FRAC)) + 1
                            while l_popped < min(lt, n_late):
                                late[l_popped]()
                                l_popped += 1
                    # defer this head's final PV + bank-B norm past the next
                    # head's/tile's first scores+exp (no PE head-block)
                    if not (last_t and h == 1):
                        def _tail(t=t, h=h, bankB=bankB, pv=pend["pv"],
                                  lt=last_t):
                            emit_pv(*pv)
                            emit_norm(t, h, bankB, 4, (3,) if lt else
                                      (0, 1, 2, 3))
                        pend["pv"] = None
                        pend["tail"] = _tail
                    else:
                        emit_pv(*pend["pv"])
                        pend["pv"] = None
                        emit_norm(t, h, bankB, 4, (3,))
                        emit_ont(t, 7)
                        for f in micro_oproj(t, 7, split_eng=True):
                            f()
                while e_popped < n_early:
                    early[e_popped]()
                    e_popped += 1
                while l_popped < n_late:
                    late[l_popped]()
                    l_popped += 1

    nc.compile()
    return nc


def _host_inputs(x, wq, wk, wv, wo):
    """Build the 8 per-core input dicts."""
    x2 = np.ascontiguousarray(x.reshape(S, D))
    xT = np.ascontiguousarray(x2.T)

    # rope pair-interleaved dh order: [0, 32, 1, 33, ...]
    perm = np.empty(DH, dtype=np.int64)
    perm[0::2] = np.arange(DH // 2)
    perm[1::2] = np.arange(DH // 2) + DH // 2

    inv_freq = 1.0 / (ROPE_THETA ** (np.arange(0, DH, 2, dtype=np.float64) / DH))
    ang = np.arange(S, dtype=np.float64)[:, None] * inv_freq[None, :]  # [S, 32]
    cosv = np.cos(ang)
    sinv = np.sin(ang)
    C64 = np.empty((DH, S), dtype=np.float32)
    Ss64 = np.empty((DH, S), dtype=np.float32)
    for j in range(DH):
        C64[j] = cosv[:, j // 2]
        Ss64[j] = sinv[:, j // 2] * (1.0 if j % 2 == 0 else -1.0)
    cosT = np.ascontiguousarray(np.tile(C64, (2, 1)))
    sinTs = np.ascontiguousarray(np.tile(Ss64, (2, 1)))

    wq4 = wq.reshape(H, DH, D)
    wk4 = wk.reshape(HKV, DH, D)
    wv4 = wv.reshape(HKV, DH, D)

    ins = []
    for c in range(NCORES):
        h0, h1 = 2 * c, 2 * c + 1
        g = h0 // (H // HKV)
        wq_c = np.concatenate([wq4[h0][perm], wq4[h1][perm]], axis=0)  # [128, D]
        wk_c = np.concatenate([wk4[g][perm], wk4[g][perm]], axis=0)    # [128, D]
        wo_c = wo[:, np.r_[h0 * DH:(h0 + 1) * DH, h1 * DH:(h1 + 1) * DH]]
        ins.append({
            "xT": xT,
            "wqT": np.ascontiguousarray(wq_c.T),
            "wkTd": np.ascontiguousarray(wk_c.T),
            "wvT": np.ascontiguousarray(wv4[g].T),
            "woT": np.ascontiguousarray(wo_c.T).astype(np.float16),
            "cosT": cosT,
            "sinTs": sinTs,
        })
    return ins


def _is_causal(mask):
    if mask.shape != (S, S):
        return False
    expected = np.where(np.tril(np.ones((S, S), dtype=bool)), np.float32(0.0),
                        np.float32(-1e9))
    return np.array_equal(mask, expected)


def run_cores(x, mask, wq, wk, wv, wo, **spmd_kwargs):
    from concourse.bass_utils import run_bass_kernel_spmd

    causal = _is_causal(np.asarray(mask))
    assert causal, "v2 fast path requires the standard causal mask"
    if True not in _cache:
        _cache[True] = _build_fast()
    nc = _cache[True]

    ins = _host_inputs(np.asarray(x), np.asarray(wq), np.asarray(wk),
                       np.asarray(wv), np.asarray(wo))
    res = run_bass_kernel_spmd(nc, ins, core_ids=list(range(NCORES)),
                               **spmd_kwargs)
    return res


def _build(causal):
    assert causal
    return _build_fast()


def kernel(x, mask, wq, wk, wv, wo):
    res = run_cores(x, mask, wq, wk, wv, wo)
    acc = np.zeros((S, D), dtype=np.float64)
    for r in res.results:
        acc += r["out"].astype(np.float64)
    return acc.astype(np.float32).reshape(B, S, D)


# revision 4
# speedup vs baseline: 1.0254x; 1.0051x over previous
"""GQA causal attention (B=1, S=4096, D=1024, H=16, HKV=4, Dh=64, RoPE) on
8 Trainium2 NeuronCores — v2 (software-pipelined single fused pass).

Sharding: 8-way head parallelism as v1 (core c owns query heads {2c, 2c+1},
sharing KV head c//2; host sums the 8 partial output projections in f64).

Device program (4 q-tiles of 1024, one TileContext):
  - Per tile t: scores S^T[k, q] per 128-key chunk (fp32r, two 512-wide
    matmuls into a double-buffered [128,1024] PSUM pair), exp on ScalarE
    (PSUM in, bf16 out, fixed bias -10 — softmax-shift-invariant), diagonal
    triangle zeroed by gpsimd affine_select. exp is the metronome: ScalarE
    runs one 0.9-1us exp per (head, chunk) and everything else is scheduled
    around keeping it saturated.
  - PV flipped: out[q(128), dh+1] accumulated per (q-chunk, k-chunk), e
    stationary, V[k, dh|ones] bf16 as 65-row moving operand (65 rows/matmul
    vs q-width in the natural orientation). Ones column -> per-partition
    softmax denominator, so normalize is reciprocal + tensor_scalar. The 8
    q-chunk accumulators live in 2 PSUM banks as interleaved accumulation
    groups (single bank-clearing start, per-element pending-zero handles
    first-write-overwrite). PV is deferred one chunk so it never blocks the
    next chunk's scores in the in-order PE queue.
  - Normalized O[q, hd] (f16) is PE-transposed to O^T per q-chunk for the
    output projection (f16 weights), staged f16, DMA'd per q-chunk row.
  - Cross-tile software pipeline: projections for tile t+1 (Q/K via
    w-stationary streams + rope; V via dh-stationary stream + PE transpose)
    and the previous tiles' output projections are split into ~0.2-0.7us
    micro-ops drained between chunk emissions, so no insertion head-blocks
    the in-order engine queues. Output projections are scheduled into the
    LATER tiles (t0->t2, t1,t2->t3) where ScalarE is the local bottleneck
    and PE has slack. The last tile's tail is normed per-q-chunk the moment
    its accumulator completes so the output tail overlaps the final chunks.
PSUM: s0,s1 (2 banks each) + oaccA,oaccB (1+1) + 2 rotating "op" banks = 8.
"""

import os

import numpy as np

B, S, D = 1, 4096, 1024
H, HKV, DH = 16, 4, 64
NCORES = 8
ROPE_THETA = 10000.0
QT = 1024
NQT = S // QT
EXP_BIAS = -10.0

_cache = {}


def _build_fast():
    import concourse.bass as bass
    import concourse.tile as tile
    from concourse import bacc, mybir
    from concourse.masks import make_identity

    f32 = mybir.dt.float32
    f32r = mybir.dt.float32r
    bf16 = mybir.dt.bfloat16
    f16 = mybir.dt.float16

    nc = bacc.Bacc(None, target_bir_lowering=False)

    xT = nc.dram_tensor("xT", [D, S], f32r, kind="ExternalInput")
    wqT = nc.dram_tensor("wqT", [D, 128], f32r, kind="ExternalInput")
    wkTd = nc.dram_tensor("wkTd", [D, 128], f32r, kind="ExternalInput")
    wvT = nc.dram_tensor("wvT", [D, DH], f32r, kind="ExternalInput")
    woT = nc.dram_tensor("woT", [128, D], f16, kind="ExternalInput")
    cosT = nc.dram_tensor("cosT", [128, S], f32, kind="ExternalInput")
    sinTs = nc.dram_tensor("sinTs", [128, S], f32, kind="ExternalInput")
    out = nc.dram_tensor("out", [S, D], f16, kind="ExternalOutput")

    with tile.TileContext(nc) as tc:
        with tc.tile_pool(name="const", bufs=1) as cpool, \
             tc.tile_pool(name="xb", bufs=2) as xb_pool, \
             tc.tile_pool(name="rtmp", bufs=2) as rtmp, \
             tc.tile_pool(name="esb", bufs=7) as e_pool, \
             tc.tile_pool(name="onp", bufs=2) as on_pool, \
             tc.tile_pool(name="ontp", bufs=4) as ont_pool, \
             tc.tile_pool(name="vts", bufs=2) as vt_pool, \
             tc.tile_pool(name="rcp", bufs=2) as rc_pool, \
             tc.tile_pool(name="txp", bufs=2) as tx_pool, \
             tc.tile_pool(name="obp", bufs=3) as ob_pool, \
             tc.tile_pool(name="psA", bufs=1, space="PSUM") as psA, \
             tc.tile_pool(name="psB", bufs=2, space="PSUM") as psB:

            # ---- resident constants ----
            wq_sb = cpool.tile([128, 8, 128], f32r)
            wk_sb = cpool.tile([128, 8, 128], f32r)
            wv_sb = cpool.tile([128, 8, DH], f32r)
            wo_sb = cpool.tile([128, D], f16)
            cos_sb = cpool.tile([128, S], f32)
            sin_sb = cpool.tile([128, S], f32)
            QTr = cpool.tile([128, S], f32r)   # rope(Q)^T rows 0-63 h0, 64-127 h1
            KTr = cpool.tile([128, S], f32r)   # rope(K)^T duplicated
            Vp = cpool.tile([128, S // 128, DH + 1], bf16)  # V[k, dh] + ones
            identb = cpool.tile([DH, DH], bf16)
            identh = cpool.tile([128, 128], f16)
            biasc = cpool.tile([128, 1], f32)

            nc.sync.dma_start(
                out=wq_sb, in_=wqT[:, :].rearrange("(c p) m -> p c m", p=128))
            nc.sync.dma_start(
                out=wk_sb, in_=wkTd[:, :].rearrange("(c p) m -> p c m", p=128))
            make_identity(nc, identb[:, :])
            make_identity(nc, identh[:, :])
            nc.vector.memset(biasc, float(EXP_BIAS))
            nc.vector.memset(Vp[:, :, DH:DH + 1], 1.0)

            xb_tiles = {}
            ON_t = {}
            ONT_t = {}
            SHUF = [i ^ 1 for i in range(32)]

            def emit_xb_dma(t, half):
                # split x by COLUMN GROUP (not d-chunk): a Q/K/V projection
                # stream for column group `half` then depends on only ONE
                # 5.8us transfer instead of two
                xb = xb_pool.tile([128, 8, 512], f32r, tag=f"xb{half}",
                                  name=f"xb_{t}_{half}")
                xb_tiles[(t, half)] = xb
                c0 = t * QT + half * 512
                src = xT[:, c0:c0 + 512]
                nc.sync.dma_start(out=xb,
                                  in_=src.rearrange("(c p) q -> p c q", p=128))

            def emit_cs_dma(t, sgh=None, eng=None):
                # sync queue: the ScalarE sequencer must stay DMA-free so it
                # can dispatch exps (DMA issue blocks the issuing SEQ on the
                # serialized HWDGE)
                eng = eng or nc.sync
                halves = (0, 1) if sgh is None else (sgh,)
                for hh in halves:
                    c0 = t * QT + hh * 512
                    sl = bass.ds(c0, 512)
                    eng.dma_start(out=cos_sb[:, sl], in_=cosT[:, c0:c0 + 512])
                    eng.dma_start(out=sin_sb[:, sl],
                                  in_=sinTs[:, c0:c0 + 512])

            def micro_qk(t, sgh, which, pool, tag):
                """Q/K projection stream + rope as a list of micro-ops."""
                sg = 2 * t + sgh
                st = {}
                w_sb = wq_sb if which == "q" else wk_sb
                dst = QTr if which == "q" else KTr
                scols = bass.ds(sg * 512, 512)

                def mk_mm(cd):
                    def f():
                        if "ps" not in st:
                            st["ps"] = pool.tile([128, 512], f32, tag=tag,
                                                 name=f"{which}t_{sg}")
                        nc.tensor.matmul(
                            st["ps"][:, :], w_sb[:, cd, :],
                            xb_tiles[(t, sgh)][:, cd, :],
                            start=(cd == 0), stop=(cd == 7))
                    return f

                def rope_a():
                    st["m1"] = rtmp.tile([128, 512], f32, tag="m1", name=f"m1_{which}_{sg}")
                    st["m2"] = rtmp.tile([128, 512], f32, tag="m2", name=f"m2_{which}_{sg}")
                    nc.vector.tensor_mul(st["m1"], st["ps"][:, :],
                                         cos_sb[:, scols])
                    nc.vector.tensor_mul(st["m2"], st["ps"][:, :],
                                         sin_sb[:, scols])

                def rope_b():
                    sh = rtmp.tile([128, 512], f32, tag="sh")
                    nc.vector.stream_shuffle(sh, st["m2"], SHUF)
                    nc.vector.tensor_add(dst[:, scols], st["m1"], sh)

                return [mk_mm(cd) for cd in range(8)] + [rope_a, rope_b]

            def micro_vt(t, sgh):
                """V projection + transpose into Vp, as micro-ops."""
                sg = 2 * t + sgh
                st = {}

                def mk_mm(cd):
                    def f():
                        if "ps" not in st:
                            st["ps"] = psB.tile([DH, 512], f32, tag="op",
                                                name=f"vt_{sg}")
                        nc.tensor.matmul(
                            st["ps"][:, :], wv_sb[:, cd, :],
                            xb_tiles[(t, sgh)][:, cd, :],
                            start=(cd == 0), stop=(cd == 7))
                    return f

                def cp():
                    st["vs"] = vt_pool.tile([DH, 512], bf16, tag="vt", name=f"vs_{sg}")
                    nc.vector.tensor_copy(st["vs"], st["ps"][:, :])

                def mk_tr(i):
                    def f():
                        kc = sg * 4 + i
                        tr = psB.tile([128, DH], bf16, tag="op",
                                      name=f"vtr_{kc}")
                        nc.tensor.transpose(
                            tr[:, :], st["vs"][:, i * 128:(i + 1) * 128],
                            identb[:, :])
                        nc.vector.tensor_copy(Vp[:, kc, 0:DH], tr[:, :])
                    return f

                return ([mk_mm(cd) for cd in range(8)] + [cp]
                        + [mk_tr(i) for i in range(4)])

            # Schraudolph-style integer exp producing bf16 directly:
            #   e = bitcast_bf16(uint16(max(A*s + B, 0)))   (~4% max rel err)
            # used to offload some exps from the saturated ScalarE onto
            # DVE (affine, PSUM read) + GpSimd (clamp + u16 convert).
            SCH_A = float(128.0 / np.log(2.0))
            SCH_B = float(127 * 128 - 4.0 + SCH_A * EXP_BIAS)
            u16 = mybir.dt.uint16

            def emit_scores_exp(t, h, c, seg=None, e=None, approx=False):
                """Scores + exp for key-chunk c over q-columns [lo, hi) of
                the tile (default: the full causal suffix)."""
                q0 = t * QT
                qs = max(0, (c - 8 * t) * 128)
                lo0, hi0 = (qs, QT) if seg is None else seg
                s_ps = psA.tile([128, QT], f32, tag=f"s{c % 2}",
                                name=f"s_{t}_{h}_{c}_{lo0}")
                lhs = KTr[64 * h:64 * h + 64, c * 128:(c + 1) * 128]
                for lo, hi in ((lo0, min(hi0, 512)), (max(lo0, 512), hi0)):
                    if lo >= hi:
                        continue
                    nc.tensor.matmul(
                        s_ps[:, bass.ds(lo, hi - lo)], lhs,
                        QTr[64 * h:64 * h + 64, q0 + lo:q0 + hi],
                        start=True, stop=True)
                if e is None:
                    e = e_pool.tile([128, QT], bf16, tag="e",
                                    name=f"e_{t}_{h}_{c}")
                if approx:
                    tx = tx_pool.tile([128, QT], f32, tag="tx",
                                      name=f"tx_{t}_{h}_{c}")
                    nc.vector.tensor_scalar(
                        tx[:, lo0:hi0], s_ps[:, lo0:hi0], SCH_A, SCH_B,
                        mybir.AluOpType.mult, mybir.AluOpType.add)
                    nc.gpsimd.tensor_scalar(
                        e[:, lo0:hi0].bitcast(u16), tx[:, lo0:hi0],
                        0.0, None, mybir.AluOpType.max)
                else:
                    nc.scalar.activation(
                        e[:, lo0:hi0], s_ps[:, lo0:hi0],
                        mybir.ActivationFunctionType.Exp,
                        bias=biasc[:, :], scale=1.0)
                if c >= 8 * t and lo0 <= qs < hi0:
                    nc.gpsimd.affine_select(
                        out=e[:, qs:qs + 128], in_=e[:, qs:qs + 128],
                        pattern=[[1, 128]],
                        compare_op=mybir.AluOpType.is_ge,
                        fill=0.0, base=0, channel_multiplier=-1)
                return e

            def emit_pv(t, c, e, bankA, bankB):
                for j in range(max(0, c - 8 * t), 8):
                    bank, jj = (bankA, j) if j < 4 else (bankB, j - 4)
                    last_c = 8 * t + (3 if j < 4 else 7)
                    nc.tensor.matmul(
                        bank[:, jj, :], e[:, j * 128:(j + 1) * 128],
                        Vp[:, c, :],
                        start=(c == 0 and jj == 0),
                        stop=(c == last_c and jj == 3))

            def emit_norm(t, h, bank, jbase, js):
                rc = rc_pool.tile([128, 4, 1], f32, tag="rc",
                                  name=f"rc_{t}_{h}_{jbase}_{js[0]}")
                j0, j1 = js[0], js[-1] + 1
                nc.vector.reciprocal_approx_fast(
                    rc[:, 0:j1 - j0, :], bank[:, j0:j1, DH:DH + 1])
                for j4 in js:
                    nc.vector.tensor_scalar_mul(
                        ON_t[t][:, jbase + j4, 64 * h:64 * h + 64],
                        bank[:, j4, 0:DH], rc[:, j4 - j0, :])

            def emit_ont(t, j, tail=False):
                tr = psB.tile([128, 128], f16, tag="op", name=f"ontr_{t}_{j}")
                nc.tensor.transpose(tr[:, :], ON_t[t][:, j, :], identh[:, :])
                if tail:  # ScalarE is idle during the kernel tail
                    nc.scalar.copy(ONT_t[t][:, j, :], tr[:, :])
                else:
                    nc.vector.tensor_copy(ONT_t[t][:, j, :], tr[:, :])

            def micro_oproj(t, j, split_eng=False):
                """Output projection for q-chunk j of tile t: two half-units.
                split_eng puts the first staging copy on ScalarE (tail mode,
                when ScalarE has gone idle)."""
                st = {}

                def half(dseg):
                    def f():
                        if "ob" not in st:
                            st["ob"] = ob_pool.tile([128, QT], f16, tag="ob", name=f"ob_{t}_{j}")
                        op = psB.tile([128, 512], f32, tag="op",
                                      name=f"op_{t}_{j}_{dseg}")
                        nc.tensor.matmul(
                            op[:, :], ONT_t[t][:, j, :],
                            wo_sb[:, dseg * 512:(dseg + 1) * 512],
                            start=True, stop=True)
                        eng = nc.scalar if (split_eng and dseg == 0) \
                            else nc.vector
                        if eng is nc.scalar:
                            eng.copy(st["ob"][:, dseg * 512:(dseg + 1) * 512],
                                     op[:, :])
                        else:
                            eng.tensor_copy(
                                st["ob"][:, dseg * 512:(dseg + 1) * 512],
                                op[:, :])
                        if dseg == 1:
                            nc.sync.dma_start(
                                out=out[t * QT + j * 128:
                                        t * QT + (j + 1) * 128, :],
                                in_=st["ob"])
                    return f

                return [half(0), half(1)]

            # ---- prologue: only what chunk 0-3's first 512 q-columns need
            # (Q/K of column group 0); the rest is emitted at the phase
            # boundary inside tile 0 so the first exps start ~15us earlier.
            # DMA issue order is deliberate: small weight/table DMAs first,
            # then the big x transfers, all on the sync queue.
            emit_cs_dma(0, sgh=0)
            emit_xb_dma(0, 0)
            emit_cs_dma(0, sgh=1)
            emit_xb_dma(0, 1)
            nc.sync.dma_start(
                out=wv_sb, in_=wvT[:, :].rearrange("(c p) m -> p c m", p=128))
            for f in micro_qk(0, 0, "q", psA, "s0"):
                f()
            for f in micro_qk(0, 0, "k", psA, "s1"):
                f()

            pend = {"pv": None, "tail": None}
            for t in range(NQT):
                NCH = 8 * (t + 1)
                last_t = t == NQT - 1
                ON_t[t] = on_pool.tile([128, 8, 128], f16, tag="ON",
                                       name=f"ON_{t}")
                if last_t:
                    ONT_t[t] = ont_pool.tile([128, 8, 128], f16, tag="ONT",
                                             name=f"ONT_{t}")
                # next tile's input DMAs first (latency-critical)
                if t + 1 < NQT:
                    emit_xb_dma(t + 1, 0)
                    emit_xb_dma(t + 1, 1)
                    emit_cs_dma(t + 1)
                if t == 0:
                    nc.sync.dma_start(out=wo_sb, in_=woT[:, :])

                # early queue: ONT transposes of t-1 + scheduled oproj units
                early = []
                if t >= 1:
                    ONT_t[t - 1] = ont_pool.tile([128, 8, 128], f16,
                                                 tag="ONT",
                                                 name=f"ONT_{t-1}")
                    for j in range(8):
                        early.append(lambda t=t, j=j: emit_ont(t - 1, j))
                # oproj schedule: t0 -> tile2, t1 and t2 -> tile3
                osrc = {2: [0], 3: [1, 2]}.get(t, [])
                for ot in osrc:
                    for j in range(8):
                        early.extend(micro_oproj(ot, j))
                # late queue: projections for tile t+1 (needs xb DMA landed)
                late = []
                if t + 1 < NQT:
                    order = ([(0, "q"), (0, "k"), (1, "q"), (1, "k")]
                             if t == 0 else
                             [(0, "q"), (0, "k"), (1, "q"), (1, "k")])
                    for sgh, which in order:
                        late.extend(micro_qk(t + 1, sgh, which, psB, "op"))
                    late.extend(micro_vt(t + 1, 0))
                    late.extend(micro_vt(t + 1, 1))

                iters = 2 * NCH
                n_early = len(early)
                n_late = len(late)
                done_iters = 0
                e_popped = l_popped = 0
                LATE_FRAC = 0.30 if t == 0 else 0.35
                for h in range(2):
                    es0 = {}
                    if t == 0 and h == 0:
                        # phase A: first 512 q-columns of chunks 0-3 need
                        # only column-group-0 Q/K (already roped) — start
                        # ScalarE while the rest of the projections build
                        for c in range(4):
                            es0[c] = emit_scores_exp(0, 0, c,
                                                     seg=(128 * c, 512))
                        # phase boundary: column-group-1 Q/K + V projections
                        for f in micro_qk(0, 1, "q", psB, "op"):
                            f()
                        for f in micro_qk(0, 1, "k", psB, "op"):
                            f()
                        for f in micro_vt(0, 0) + micro_vt(0, 1):
                            f()
                    bankA = psA.tile([128, 4, DH + 1], f32, tag="oaccA",
                                     name=f"oA_{t}_{h}")
                    bankB = psA.tile([128, 4, DH + 1], f32, tag="oaccB",
                                     name=f"oB_{t}_{h}")
                    for c in range(NCH):
                        off = False  # ScalarE->DVE/GpSimd exp offload: net loss (queue serialization)
                        if c in es0:
                            e = emit_scores_exp(t, h, c, seg=(512, QT),
                                                e=es0[c])
                        else:
                            e = emit_scores_exp(t, h, c, approx=off)
                        if pend["pv"] is not None:
                            emit_pv(*pend["pv"])
                            pend["pv"] = None
                        if pend["tail"] is not None:
                            pend["tail"]()
                            pend["tail"] = None
                        pend["pv"] = (t, c, e, bankA, bankB)
                        cj = c - 8 * t
                        if cj == 4:
                            emit_norm(t, h, bankA, 0, (0, 1, 2, 3))
                            if last_t and h == 1:
                                for j in range(4):
                                    emit_ont(t, j)
                                    for f in micro_oproj(t, j,
                                                         split_eng=True):
                                        f()
                        if last_t and cj >= 5:
                            jd = cj - 1
                            emit_norm(t, h, bankB, 4, (jd - 4,))
                            if h == 1:
                                emit_ont(t, jd)
                                for f in micro_oproj(t, jd, split_eng=True):
                                    f()
                        done_iters += 1
                        et = (n_early * done_iters * 4 + 3 * iters) \
                            // (3 * iters)
                        while e_popped < min(et, n_early):
                            early[e_popped]()
                            e_popped += 1
                        prog = done_iters / iters
                        if prog > LATE_FRAC:
                            lt = int(n_late * (prog - LATE_FRAC)
                                     / (0.95 - LATE_FRAC)) + 1
                            while l_popped < min(lt, n_late):
                                late[l_popped]()
                                l_popped += 1
                    # defer this head's final PV + bank-B norm past the next
                    # head's/tile's first scores+exp (no PE head-block)
                    if not (last_t and h == 1):
                        def _tail(t=t, h=h, bankB=bankB, pv=pend["pv"],
                                  lt=last_t):
                            emit_pv(*pv)
                            emit_norm(t, h, bankB, 4, (3,) if lt else
                                      (0, 1, 2, 3))
                        pend["pv"] = None
                        pend["tail"] = _tail
                    else:
                        emit_pv(*pend["pv"])
                        pend["pv"] = None
                        emit_norm(t, h, bankB, 4, (3,))
                        emit_ont(t, 7)
                        for f in micro_oproj(t, 7, split_eng=True):
                            f()
                while e_popped < n_early:
                    early[e_popped]()
                    e_popped += 1
                while l_popped < n_late:
                    late[l_popped]()
                    l_popped += 1

    nc.compile()
    return nc


def _host_inputs(x, wq, wk, wv, wo):
    """Build the 8 per-core input dicts."""
    x2 = np.ascontiguousarray(x.reshape(S, D))
    xT = np.ascontiguousarray(x2.T)

    # rope pair-interleaved dh order: [0, 32, 1, 33, ...]
    perm = np.empty(DH, dtype=np.int64)
    perm[0::2] = np.arange(DH // 2)
    perm[1::2] = np.arange(DH // 2) + DH // 2

    inv_freq = 1.0 / (ROPE_THETA ** (np.arange(0, DH, 2, dtype=np.float64) / DH))
    ang = np.arange(S, dtype=np.float64)[:, None] * inv_freq[None, :]  # [S, 32]
    cosv = np.cos(ang)
    sinv = np.sin(ang)
    C64 = np.empty((DH, S), dtype=np.float32)
    Ss64 = np.empty((DH, S), dtype=np.float32)
    for j in range(DH):
        C64[j] = cosv[:, j // 2]
        Ss64[j] = sinv[:, j // 2] * (1.0 if j % 2 == 0 else -1.0)
    cosT = np.ascontiguousarray(np.tile(C64, (2, 1)))
    sinTs = np.ascontiguousarray(np.tile(Ss64, (2, 1)))

    wq4 = wq.reshape(H, DH, D)
    wk4 = wk.reshape(HKV, DH, D)
    wv4 = wv.reshape(HKV, DH, D)

    ins = []
    for c in range(NCORES):
        h0, h1 = 2 * c, 2 * c + 1
        g = h0 // (H // HKV)
        wq_c = np.concatenate([wq4[h0][perm], wq4[h1][perm]], axis=0)  # [128, D]
        wk_c = np.concatenate([wk4[g][perm], wk4[g][perm]], axis=0)    # [128, D]
        wo_c = wo[:, np.r_[h0 * DH:(h0 + 1) * DH, h1 * DH:(h1 + 1) * DH]]
        ins.append({
            "xT": xT,
            "wqT": np.ascontiguousarray(wq_c.T),
            "wkTd": np.ascontiguousarray(wk_c.T),
            "wvT": np.ascontiguousarray(wv4[g].T),
            "woT": np.ascontiguousarray(wo_c.T).astype(np.float16),
            "cosT": cosT,
            "sinTs": sinTs,
        })
    return ins


def _is_causal(mask):
    if mask.shape != (S, S):
        return False
    expected = np.where(np.tril(np.ones((S, S), dtype=bool)), np.float32(0.0),
                        np.float32(-1e9))
    return np.array_equal(mask, expected)


def run_cores(x, mask, wq, wk, wv, wo, **spmd_kwargs):
    from concourse.bass_utils import run_bass_kernel_spmd

    causal = _is_causal(np.asarray(mask))
    assert causal, "v2 fast path requires the standard causal mask"
    if True not in _cache:
        _cache[True] = _build_fast()
    nc = _cache[True]

    ins = _host_inputs(np.asarray(x), np.asarray(wq), np.asarray(wk),
                       np.asarray(wv), np.asarray(wo))
    res = run_bass_kernel_spmd(nc, ins, core_ids=list(range(NCORES)),
                               **spmd_kwargs)
    return res


def _build(causal):
    assert causal
    return _build_fast()


def kernel(x, mask, wq, wk, wv, wo):
    res = run_cores(x, mask, wq, wk, wv, wo)
    acc = np.zeros((S, D), dtype=np.float64)
    for r in res.results:
        acc += r["out"].astype(np.float64)
    return acc.astype(np.float32).reshape(B, S, D)


# revision 5
# speedup vs baseline: 1.0321x; 1.0065x over previous
"""GQA causal attention (B=1, S=4096, D=1024, H=16, HKV=4, Dh=64, RoPE) on
8 Trainium2 NeuronCores — v2 (software-pipelined single fused pass).

Sharding: 8-way head parallelism as v1 (core c owns query heads {2c, 2c+1},
sharing KV head c//2; host sums the 8 partial output projections in f64).

Device program (4 q-tiles of 1024, one TileContext):
  - Per tile t: scores S^T[k, q] per 128-key chunk (fp32r, two 512-wide
    matmuls into a double-buffered [128,1024] PSUM pair), exp on ScalarE
    (PSUM in, bf16 out, fixed bias -10 — softmax-shift-invariant), diagonal
    triangle zeroed by gpsimd affine_select. exp is the metronome: ScalarE
    runs one 0.9-1us exp per (head, chunk) and everything else is scheduled
    around keeping it saturated.
  - PV flipped: out[q(128), dh+1] accumulated per (q-chunk, k-chunk), e
    stationary, V[k, dh|ones] bf16 as 65-row moving operand (65 rows/matmul
    vs q-width in the natural orientation). Ones column -> per-partition
    softmax denominator, so normalize is reciprocal + tensor_scalar. The 8
    q-chunk accumulators live in 2 PSUM banks as interleaved accumulation
    groups (single bank-clearing start, per-element pending-zero handles
    first-write-overwrite). PV is deferred one chunk so it never blocks the
    next chunk's scores in the in-order PE queue.
  - Normalized O[q, hd] (f16) is PE-transposed to O^T per q-chunk for the
    output projection (f16 weights), staged f16, DMA'd per q-chunk row.
  - Cross-tile software pipeline: projections for tile t+1 (Q/K via
    w-stationary streams + rope; V via dh-stationary stream + PE transpose)
    and the previous tiles' output projections are split into ~0.2-0.7us
    micro-ops drained between chunk emissions, so no insertion head-blocks
    the in-order engine queues. Output projections are scheduled into the
    LATER tiles (t0->t2, t1,t2->t3) where ScalarE is the local bottleneck
    and PE has slack. The last tile's tail is normed per-q-chunk the moment
    its accumulator completes so the output tail overlaps the final chunks.
PSUM: s0,s1 (2 banks each) + oaccA,oaccB (1+1) + 2 rotating "op" banks = 8.
"""

import os

import numpy as np

B, S, D = 1, 4096, 1024
H, HKV, DH = 16, 4, 64
NCORES = 8
ROPE_THETA = 10000.0
QT = 1024
NQT = S // QT
EXP_BIAS = -10.0

_cache = {}


def _build_fast():
    import concourse.bass as bass
    import concourse.tile as tile
    from concourse import bacc, mybir
    from concourse.masks import make_identity

    f32 = mybir.dt.float32
    f32r = mybir.dt.float32r
    bf16 = mybir.dt.bfloat16
    f16 = mybir.dt.float16

    nc = bacc.Bacc(None, target_bir_lowering=False)

    xT = nc.dram_tensor("xT", [D, S], f32r, kind="ExternalInput")
    wqT = nc.dram_tensor("wqT", [D, 128], f32r, kind="ExternalInput")
    wkTd = nc.dram_tensor("wkTd", [D, 128], f32r, kind="ExternalInput")
    wvT = nc.dram_tensor("wvT", [D, DH], f32r, kind="ExternalInput")
    woT = nc.dram_tensor("woT", [128, D], f16, kind="ExternalInput")
    cosT = nc.dram_tensor("cosT", [128, S], f32, kind="ExternalInput")
    sinTs = nc.dram_tensor("sinTs", [128, S], f32, kind="ExternalInput")
    out = nc.dram_tensor("out", [S, D], f16, kind="ExternalOutput")

    with tile.TileContext(nc) as tc:
        with tc.tile_pool(name="const", bufs=1) as cpool, \
             tc.tile_pool(name="xb", bufs=2) as xb_pool, \
             tc.tile_pool(name="rtmp", bufs=3) as rtmp, \
             tc.tile_pool(name="esb", bufs=7) as e_pool, \
             tc.tile_pool(name="onp", bufs=2) as on_pool, \
             tc.tile_pool(name="ontp", bufs=4) as ont_pool, \
             tc.tile_pool(name="vts", bufs=3) as vt_pool, \
             tc.tile_pool(name="rcp", bufs=2) as rc_pool, \
             tc.tile_pool(name="txp", bufs=2) as tx_pool, \
             tc.tile_pool(name="obp", bufs=4) as ob_pool, \
             tc.tile_pool(name="psA", bufs=1, space="PSUM") as psA, \
             tc.tile_pool(name="psB", bufs=2, space="PSUM") as psB:

            # ---- resident constants ----
            wq_sb = cpool.tile([128, 8, 128], f32r)
            wk_sb = cpool.tile([128, 8, 128], f32r)
            wv_sb = cpool.tile([128, 8, DH], f32r)
            wo_sb = cpool.tile([128, D], f16)
            cos_sb = cpool.tile([128, S], f32)
            sin_sb = cpool.tile([128, S], f32)
            QTr = cpool.tile([128, S], f32r)   # rope(Q)^T rows 0-63 h0, 64-127 h1
            KTr = cpool.tile([128, S], f32r)   # rope(K)^T duplicated
            Vp = cpool.tile([128, S // 128, DH + 1], bf16)  # V[k, dh] + ones
            identb = cpool.tile([DH, DH], bf16)
            identh = cpool.tile([128, 128], f16)
            biasc = cpool.tile([128, 1], f32)

            nc.sync.dma_start(
                out=wq_sb, in_=wqT[:, :].rearrange("(c p) m -> p c m", p=128))
            nc.sync.dma_start(
                out=wk_sb, in_=wkTd[:, :].rearrange("(c p) m -> p c m", p=128))
            make_identity(nc, identb[:, :])
            make_identity(nc, identh[:, :])
            nc.vector.memset(biasc, float(EXP_BIAS))
            nc.vector.memset(Vp[:, :, DH:DH + 1], 1.0)

            xb_tiles = {}
            ON_t = {}
            ONT_t = {}
            SHUF = [i ^ 1 for i in range(32)]

            def emit_xb_dma(t, half):
                # split x by COLUMN GROUP (not d-chunk): a Q/K/V projection
                # stream for column group `half` then depends on only ONE
                # 5.8us transfer instead of two
                xb = xb_pool.tile([128, 8, 512], f32r, tag=f"xb{half}",
                                  name=f"xb_{t}_{half}")
                xb_tiles[(t, half)] = xb
                c0 = t * QT + half * 512
                src = xT[:, c0:c0 + 512]
                nc.sync.dma_start(out=xb,
                                  in_=src.rearrange("(c p) q -> p c q", p=128))

            def emit_cs_dma(t, sgh=None, eng=None):
                # sync queue: the ScalarE sequencer must stay DMA-free so it
                # can dispatch exps (DMA issue blocks the issuing SEQ on the
                # serialized HWDGE)
                eng = eng or nc.sync
                halves = (0, 1) if sgh is None else (sgh,)
                for hh in halves:
                    c0 = t * QT + hh * 512
                    sl = bass.ds(c0, 512)
                    eng.dma_start(out=cos_sb[:, sl], in_=cosT[:, c0:c0 + 512])
                    eng.dma_start(out=sin_sb[:, sl],
                                  in_=sinTs[:, c0:c0 + 512])

            def micro_qk(t, sgh, which, pool, tag):
                """Q/K projection stream + rope as a list of micro-ops."""
                sg = 2 * t + sgh
                st = {}
                w_sb = wq_sb if which == "q" else wk_sb
                dst = QTr if which == "q" else KTr
                scols = bass.ds(sg * 512, 512)

                def mk_mm(cd):
                    def f():
                        if "ps" not in st:
                            st["ps"] = pool.tile([128, 512], f32, tag=tag,
                                                 name=f"{which}t_{sg}")
                        nc.tensor.matmul(
                            st["ps"][:, :], w_sb[:, cd, :],
                            xb_tiles[(t, sgh)][:, cd, :],
                            start=(cd == 0), stop=(cd == 7))
                    return f

                def rope_a():
                    st["m1"] = rtmp.tile([128, 512], f32, tag="m1", name=f"m1_{which}_{sg}")
                    st["m2"] = rtmp.tile([128, 512], f32, tag="m2", name=f"m2_{which}_{sg}")
                    nc.vector.tensor_mul(st["m1"], st["ps"][:, :],
                                         cos_sb[:, scols])
                    nc.vector.tensor_mul(st["m2"], st["ps"][:, :],
                                         sin_sb[:, scols])

                def rope_b():
                    sh = rtmp.tile([128, 512], f32, tag="sh")
                    nc.vector.stream_shuffle(sh, st["m2"], SHUF)
                    nc.vector.tensor_add(dst[:, scols], st["m1"], sh)

                return [mk_mm(cd) for cd in range(8)] + [rope_a, rope_b]

            def micro_vt(t, sgh):
                """V projection + transpose into Vp, as micro-ops."""
                sg = 2 * t + sgh
                st = {}

                def mk_mm(cd):
                    def f():
                        if "ps" not in st:
                            st["ps"] = psB.tile([DH, 512], f32, tag="op",
                                                name=f"vt_{sg}")
                        nc.tensor.matmul(
                            st["ps"][:, :], wv_sb[:, cd, :],
                            xb_tiles[(t, sgh)][:, cd, :],
                            start=(cd == 0), stop=(cd == 7))
                    return f

                def cp():
                    st["vs"] = vt_pool.tile([DH, 512], bf16, tag="vt", name=f"vs_{sg}")
                    nc.vector.tensor_copy(st["vs"], st["ps"][:, :])

                def mk_tr(i):
                    def f():
                        kc = sg * 4 + i
                        tr = psB.tile([128, DH], bf16, tag="op",
                                      name=f"vtr_{kc}")
                        nc.tensor.transpose(
                            tr[:, :], st["vs"][:, i * 128:(i + 1) * 128],
                            identb[:, :])
                        nc.vector.tensor_copy(Vp[:, kc, 0:DH], tr[:, :])
                    return f

                return ([mk_mm(cd) for cd in range(8)] + [cp]
                        + [mk_tr(i) for i in range(4)])

            # Schraudolph-style integer exp producing bf16 directly:
            #   e = bitcast_bf16(uint16(max(A*s + B, 0)))   (~4% max rel err)
            # used to offload some exps from the saturated ScalarE onto
            # DVE (affine, PSUM read) + GpSimd (clamp + u16 convert).
            SCH_A = float(128.0 / np.log(2.0))
            SCH_B = float(127 * 128 - 4.0 + SCH_A * EXP_BIAS)
            u16 = mybir.dt.uint16

            def emit_scores_exp(t, h, c, seg=None, e=None, approx=False):
                """Scores + exp for key-chunk c over q-columns [lo, hi) of
                the tile (default: the full causal suffix)."""
                q0 = t * QT
                qs = max(0, (c - 8 * t) * 128)
                lo0, hi0 = (qs, QT) if seg is None else seg
                s_ps = psA.tile([128, QT], f32, tag=f"s{c % 2}",
                                name=f"s_{t}_{h}_{c}_{lo0}")
                lhs = KTr[64 * h:64 * h + 64, c * 128:(c + 1) * 128]
                for lo, hi in ((lo0, min(hi0, 512)), (max(lo0, 512), hi0)):
                    if lo >= hi:
                        continue
                    nc.tensor.matmul(
                        s_ps[:, bass.ds(lo, hi - lo)], lhs,
                        QTr[64 * h:64 * h + 64, q0 + lo:q0 + hi],
                        start=True, stop=True)
                if e is None:
                    e = e_pool.tile([128, QT], bf16, tag="e",
                                    name=f"e_{t}_{h}_{c}")
                if approx:
                    tx = tx_pool.tile([128, QT], f32, tag="tx",
                                      name=f"tx_{t}_{h}_{c}")
                    nc.vector.tensor_scalar(
                        tx[:, lo0:hi0], s_ps[:, lo0:hi0], SCH_A, SCH_B,
                        mybir.AluOpType.mult, mybir.AluOpType.add)
                    nc.gpsimd.tensor_scalar(
                        e[:, lo0:hi0].bitcast(u16), tx[:, lo0:hi0],
                        0.0, None, mybir.AluOpType.max)
                else:
                    nc.scalar.activation(
                        e[:, lo0:hi0], s_ps[:, lo0:hi0],
                        mybir.ActivationFunctionType.Exp,
                        bias=biasc[:, :], scale=1.0)
                if c >= 8 * t and lo0 <= qs < hi0:
                    nc.gpsimd.affine_select(
                        out=e[:, qs:qs + 128], in_=e[:, qs:qs + 128],
                        pattern=[[1, 128]],
                        compare_op=mybir.AluOpType.is_ge,
                        fill=0.0, base=0, channel_multiplier=-1)
                return e

            def emit_pv(t, c, e, bankA, bankB):
                for j in range(max(0, c - 8 * t), 8):
                    bank, jj = (bankA, j) if j < 4 else (bankB, j - 4)
                    last_c = 8 * t + (3 if j < 4 else 7)
                    nc.tensor.matmul(
                        bank[:, jj, :], e[:, j * 128:(j + 1) * 128],
                        Vp[:, c, :],
                        start=(c == 0 and jj == 0),
                        stop=(c == last_c and jj == 3))

            def emit_norm(t, h, bank, jbase, js):
                rc = rc_pool.tile([128, 4, 1], f32, tag="rc",
                                  name=f"rc_{t}_{h}_{jbase}_{js[0]}")
                j0, j1 = js[0], js[-1] + 1
                nc.vector.reciprocal_approx_fast(
                    rc[:, 0:j1 - j0, :], bank[:, j0:j1, DH:DH + 1])
                for j4 in js:
                    nc.vector.tensor_scalar_mul(
                        ON_t[t][:, jbase + j4, 64 * h:64 * h + 64],
                        bank[:, j4, 0:DH], rc[:, j4 - j0, :])

            def emit_ont(t, j, tail=False):
                tr = psB.tile([128, 128], f16, tag="op", name=f"ontr_{t}_{j}")
                nc.tensor.transpose(tr[:, :], ON_t[t][:, j, :], identh[:, :])
                if tail:  # ScalarE is idle during the kernel tail
                    nc.scalar.copy(ONT_t[t][:, j, :], tr[:, :])
                else:
                    nc.vector.tensor_copy(ONT_t[t][:, j, :], tr[:, :])

            def micro_oproj(t, j, split_eng=False):
                """Output projection for q-chunk j of tile t: two half-units.
                split_eng puts the first staging copy on ScalarE (tail mode,
                when ScalarE has gone idle)."""
                st = {}

                def half(dseg):
                    def f():
                        if "ob" not in st:
                            st["ob"] = ob_pool.tile([128, QT], f16, tag="ob", name=f"ob_{t}_{j}")
                        op = psB.tile([128, 512], f32, tag="op",
                                      name=f"op_{t}_{j}_{dseg}")
                        nc.tensor.matmul(
                            op[:, :], ONT_t[t][:, j, :],
                            wo_sb[:, dseg * 512:(dseg + 1) * 512],
                            start=True, stop=True)
                        eng = nc.scalar if (split_eng and dseg == 0) \
                            else nc.vector
                        if eng is nc.scalar:
                            eng.copy(st["ob"][:, dseg * 512:(dseg + 1) * 512],
                                     op[:, :])
                        else:
                            eng.tensor_copy(
                                st["ob"][:, dseg * 512:(dseg + 1) * 512],
                                op[:, :])
                        if dseg == 1:
                            nc.sync.dma_start(
                                out=out[t * QT + j * 128:
                                        t * QT + (j + 1) * 128, :],
                                in_=st["ob"])
                    return f

                return [half(0), half(1)]

            # ---- prologue: only what chunk 0-3's first 512 q-columns need
            # (Q/K of column group 0); the rest is emitted at the phase
            # boundary inside tile 0 so the first exps start ~15us earlier.
            # DMA issue order is deliberate: small weight/table DMAs first,
            # then the big x transfers, all on the sync queue.
            emit_cs_dma(0, sgh=0)
            emit_xb_dma(0, 0)
            emit_cs_dma(0, sgh=1)
            emit_xb_dma(0, 1)
            nc.sync.dma_start(
                out=wv_sb, in_=wvT[:, :].rearrange("(c p) m -> p c m", p=128))
            for f in micro_qk(0, 0, "q", psA, "s0"):
                f()
            for f in micro_qk(0, 0, "k", psA, "s1"):
                f()

            pend = {"pv": None, "tail": None}
            for t in range(NQT):
                NCH = 8 * (t + 1)
                last_t = t == NQT - 1
                ON_t[t] = on_pool.tile([128, 8, 128], f16, tag="ON",
                                       name=f"ON_{t}")
                if last_t:
                    ONT_t[t] = ont_pool.tile([128, 8, 128], f16, tag="ONT",
                                             name=f"ONT_{t}")
                # next tile's input DMAs first (latency-critical)
                if t + 1 < NQT:
                    emit_xb_dma(t + 1, 0)
                    emit_xb_dma(t + 1, 1)
                    emit_cs_dma(t + 1)
                if t == 0:
                    nc.sync.dma_start(out=wo_sb, in_=woT[:, :])

                # early queue: ONT transposes of t-1 + scheduled oproj units
                early = []
                if t >= 1:
                    ONT_t[t - 1] = ont_pool.tile([128, 8, 128], f16,
                                                 tag="ONT",
                                                 name=f"ONT_{t-1}")
                    for j in range(8):
                        early.append(lambda t=t, j=j: emit_ont(t - 1, j))
                # oproj schedule: t0 -> tile2, t1 and t2 -> tile3
                osrc = {2: [0], 3: [1, 2]}.get(t, [])
                for ot in osrc:
                    for j in range(8):
                        early.extend(micro_oproj(ot, j))
                # late queue: projections for tile t+1 (needs xb DMA landed)
                late = []
                if t + 1 < NQT:
                    order = ([(0, "q"), (0, "k"), (1, "q"), (1, "k")]
                             if t == 0 else
                             [(0, "q"), (0, "k"), (1, "q"), (1, "k")])
                    for sgh, which in order:
                        late.extend(micro_qk(t + 1, sgh, which, psB, "op"))
                    late.extend(micro_vt(t + 1, 0))
                    late.extend(micro_vt(t + 1, 1))

                iters = 2 * NCH
                n_early = len(early)
                n_late = len(late)
                done_iters = 0
                e_popped = l_popped = 0
                LATE_FRAC = 0.30 if t == 0 else 0.35
                for h in range(2):
                    es0 = {}
                    if t == 0 and h == 0:
                        # phase A: first 512 q-columns of chunks 0-3 need
                        # only column-group-0 Q/K (already roped) — start
                        # ScalarE while the rest of the projections build
                        for c in range(4):
                            es0[c] = emit_scores_exp(0, 0, c,
                                                     seg=(128 * c, 512))
                        # phase boundary: column-group-1 Q/K + V projections
                        for f in micro_qk(0, 1, "q", psB, "op"):
                            f()
                        for f in micro_qk(0, 1, "k", psB, "op"):
                            f()
                        for f in micro_vt(0, 0) + micro_vt(0, 1):
                            f()
                    bankA = psA.tile([128, 4, DH + 1], f32, tag="oaccA",
                                     name=f"oA_{t}_{h}")
                    bankB = psA.tile([128, 4, DH + 1], f32, tag="oaccB",
                                     name=f"oB_{t}_{h}")
                    for c in range(NCH):
                        off = False  # ScalarE->DVE/GpSimd exp offload: net loss (queue serialization)
                        if c in es0:
                            e = emit_scores_exp(t, h, c, seg=(512, QT),
                                                e=es0[c])
                        else:
                            e = emit_scores_exp(t, h, c, approx=off)
                        if pend["pv"] is not None:
                            emit_pv(*pend["pv"])
                            pend["pv"] = None
                        if pend["tail"] is not None:
                            pend["tail"]()
                            pend["tail"] = None
                        pend["pv"] = (t, c, e, bankA, bankB)
                        cj = c - 8 * t
                        if cj == 4:
                            emit_norm(t, h, bankA, 0, (0, 1, 2, 3))
                            if last_t and h == 1:
                                for j in range(4):
                                    emit_ont(t, j)
                                    for f in micro_oproj(t, j,
                                                         split_eng=True):
                                        f()
                        if last_t and cj >= 5:
                            jd = cj - 1
                            emit_norm(t, h, bankB, 4, (jd - 4,))
                            if h == 1:
                                emit_ont(t, jd)
                                for f in micro_oproj(t, jd, split_eng=True):
                                    f()
                        done_iters += 1
                        et = (n_early * done_iters * 4 + 3 * iters) \
                            // (3 * iters)
                        while e_popped < min(et, n_early):
                            early[e_popped]()
                            e_popped += 1
                        prog = done_iters / iters
                        if prog > LATE_FRAC:
                            lt = int(n_late * (prog - LATE_FRAC)
                                     / (0.95 - LATE_FRAC)) + 1
                            while l_popped < min(lt, n_late):
                                late[l_popped]()
                                l_popped += 1
                    # defer this head's final PV + bank-B norm past the next
                    # head's/tile's first scores+exp (no PE head-block)
                    if not (last_t and h == 1):
                        def _tail(t=t, h=h, bankB=bankB, pv=pend["pv"],
                                  lt=last_t):
                            emit_pv(*pv)
                            emit_norm(t, h, bankB, 4, (3,) if lt else
                                      (0, 1, 2, 3))
                        pend["pv"] = None
                        pend["tail"] = _tail
                    else:
                        emit_pv(*pend["pv"])
                        pend["pv"] = None
                        emit_norm(t, h, bankB, 4, (3,))
                        emit_ont(t, 7)
                        for f in micro_oproj(t, 7, split_eng=True):
                            f()
                while e_popped < n_early:
                    early[e_popped]()
                    e_popped += 1
                while l_popped < n_late:
                    late[l_popped]()
                    l_popped += 1

    nc.compile()
    return nc


def _host_inputs(x, wq, wk, wv, wo):
    """Build the 8 per-core input dicts."""
    x2 = np.ascontiguousarray(x.reshape(S, D))
    xT = np.ascontiguousarray(x2.T)

    # rope pair-interleaved dh order: [0, 32, 1, 33, ...]
    perm = np.empty(DH, dtype=np.int64)
    perm[0::2] = np.arange(DH // 2)
    perm[1::2] = np.arange(DH // 2) + DH // 2

    inv_freq = 1.0 / (ROPE_THETA ** (np.arange(0, DH, 2, dtype=np.float64) / DH))
    ang = np.arange(S, dtype=np.float64)[:, None] * inv_freq[None, :]  # [S, 32]
    cosv = np.cos(ang)
    sinv = np.sin(ang)
    C64 = np.empty((DH, S), dtype=np.float32)
    Ss64 = np.empty((DH, S), dtype=np.float32)
    for j in range(DH):
        C64[j] = cosv[:, j // 2]
        Ss64[j] = sinv[:, j // 2] * (1.0 if j % 2 == 0 else -1.0)
    cosT = np.ascontiguousarray(np.tile(C64, (2, 1)))
    sinTs = np.ascontiguousarray(np.tile(Ss64, (2, 1)))

    wq4 = wq.reshape(H, DH, D)
    wk4 = wk.reshape(HKV, DH, D)
    wv4 = wv.reshape(HKV, DH, D)

    ins = []
    for c in range(NCORES):
        h0, h1 = 2 * c, 2 * c + 1
        g = h0 // (H // HKV)
        wq_c = np.concatenate([wq4[h0][perm], wq4[h1][perm]], axis=0)  # [128, D]
        wk_c = np.concatenate([wk4[g][perm], wk4[g][perm]], axis=0)    # [128, D]
        wo_c = wo[:, np.r_[h0 * DH:(h0 + 1) * DH, h1 * DH:(h1 + 1) * DH]]
        ins.append({
            "xT": xT,
            "wqT": np.ascontiguousarray(wq_c.T),
            "wkTd": np.ascontiguousarray(wk_c.T),
            "wvT": np.ascontiguousarray(wv4[g].T),
            "woT": np.ascontiguousarray(wo_c.T).astype(np.float16),
            "cosT": cosT,
            "sinTs": sinTs,
        })
    return ins


def _is_causal(mask):
    if mask.shape != (S, S):
        return False
    expected = np.where(np.tril(np.ones((S, S), dtype=bool)), np.float32(0.0),
                        np.float32(-1e9))
    return np.array_equal(mask, expected)


def run_cores(x, mask, wq, wk, wv, wo, **spmd_kwargs):
    from concourse.bass_utils import run_bass_kernel_spmd

    causal = _is_causal(np.asarray(mask))
    assert causal, "v2 fast path requires the standard causal mask"
    if True not in _cache:
        _cache[True] = _build_fast()
    nc = _cache[True]

    ins = _host_inputs(np.asarray(x), np.asarray(wq), np.asarray(wk),
                       np.asarray(wv), np.asarray(wo))
    res = run_bass_kernel_spmd(nc, ins, core_ids=list(range(NCORES)),
                               **spmd_kwargs)
    return res


def _build(causal):
    assert causal
    return _build_fast()


def kernel(x, mask, wq, wk, wv, wo):
    res = run_cores(x, mask, wq, wk, wv, wo)
    acc = np.zeros((S, D), dtype=np.float64)
    for r in res.results:
        acc += r["out"].astype(np.float64)
    return acc.astype(np.float32).reshape(B, S, D)


# revision 6
# speedup vs baseline: 1.0367x; 1.0044x over previous
"""GQA causal attention (B=1, S=4096, D=1024, H=16, HKV=4, Dh=64, RoPE) on
8 Trainium2 NeuronCores — v2 (software-pipelined single fused pass).

Sharding: 8-way head parallelism as v1 (core c owns query heads {2c, 2c+1},
sharing KV head c//2; host sums the 8 partial output projections in f64).

Device program (4 q-tiles of 1024, one TileContext):
  - Per tile t: scores S^T[k, q] per 128-key chunk (fp32r, two 512-wide
    matmuls into a double-buffered [128,1024] PSUM pair), exp on ScalarE
    (PSUM in, bf16 out, fixed bias -10 — softmax-shift-invariant), diagonal
    triangle zeroed by gpsimd affine_select. exp is the metronome: ScalarE
    runs one 0.9-1us exp per (head, chunk) and everything else is scheduled
    around keeping it saturated.
  - PV flipped: out[q(128), dh+1] accumulated per (q-chunk, k-chunk), e
    stationary, V[k, dh|ones] bf16 as 65-row moving operand (65 rows/matmul
    vs q-width in the natural orientation). Ones column -> per-partition
    softmax denominator, so normalize is reciprocal + tensor_scalar. The 8
    q-chunk accumulators live in 2 PSUM banks as interleaved accumulation
    groups (single bank-clearing start, per-element pending-zero handles
    first-write-overwrite). PV is deferred one chunk so it never blocks the
    next chunk's scores in the in-order PE queue.
  - Normalized O[q, hd] (f16) is PE-transposed to O^T per q-chunk for the
    output projection (f16 weights), staged f16, DMA'd per q-chunk row.
  - Cross-tile software pipeline: projections for tile t+1 (Q/K via
    w-stationary streams + rope; V via dh-stationary stream + PE transpose)
    and the previous tiles' output projections are split into ~0.2-0.7us
    micro-ops drained between chunk emissions, so no insertion head-blocks
    the in-order engine queues. Output projections are scheduled into the
    LATER tiles (t0->t2, t1,t2->t3) where ScalarE is the local bottleneck
    and PE has slack. The last tile's tail is normed per-q-chunk the moment
    its accumulator completes so the output tail overlaps the final chunks.
PSUM: s0,s1 (2 banks each) + oaccA,oaccB (1+1) + 2 rotating "op" banks = 8.
"""

import os

import numpy as np

B, S, D = 1, 4096, 1024
H, HKV, DH = 16, 4, 64
NCORES = 8
ROPE_THETA = 10000.0
QT = 1024
NQT = S // QT
EXP_BIAS = -10.0

_cache = {}


def _build_fast():
    import concourse.bass as bass
    import concourse.tile as tile
    from concourse import bacc, mybir
    from concourse.masks import make_identity

    f32 = mybir.dt.float32
    f32r = mybir.dt.float32r
    bf16 = mybir.dt.bfloat16
    f16 = mybir.dt.float16

    nc = bacc.Bacc(None, target_bir_lowering=False)

    xT = nc.dram_tensor("xT", [D, S], f32r, kind="ExternalInput")
    wqT = nc.dram_tensor("wqT", [D, 128], f32r, kind="ExternalInput")
    wkTd = nc.dram_tensor("wkTd", [D, 128], f32r, kind="ExternalInput")
    wvT = nc.dram_tensor("wvT", [D, DH], f32r, kind="ExternalInput")
    woT = nc.dram_tensor("woT", [128, D], f16, kind="ExternalInput")
    cosT = nc.dram_tensor("cosT", [128, S], f32, kind="ExternalInput")
    sinTs = nc.dram_tensor("sinTs", [128, S], f32, kind="ExternalInput")
    out = nc.dram_tensor("out", [S, D], f16, kind="ExternalOutput")

    with tile.TileContext(nc) as tc:
        with tc.tile_pool(name="const", bufs=1) as cpool, \
             tc.tile_pool(name="xb", bufs=2) as xb_pool, \
             tc.tile_pool(name="rtmp", bufs=4) as rtmp, \
             tc.tile_pool(name="esb", bufs=7) as e_pool, \
             tc.tile_pool(name="onp", bufs=2) as on_pool, \
             tc.tile_pool(name="ontp", bufs=4) as ont_pool, \
             tc.tile_pool(name="vts", bufs=3) as vt_pool, \
             tc.tile_pool(name="rcp", bufs=2) as rc_pool, \
             tc.tile_pool(name="txp", bufs=2) as tx_pool, \
             tc.tile_pool(name="obp", bufs=4) as ob_pool, \
             tc.tile_pool(name="psA", bufs=1, space="PSUM") as psA, \
             tc.tile_pool(name="psB", bufs=2, space="PSUM") as psB:

            # ---- resident constants ----
            wq_sb = cpool.tile([128, 8, 128], f32r)
            wk_sb = cpool.tile([128, 8, 128], f32r)
            wv_sb = cpool.tile([128, 8, DH], f32r)
            wo_sb = cpool.tile([128, D], f16)
            cos_sb = cpool.tile([128, S], f32)
            sin_sb = cpool.tile([128, S], f32)
            QTr = cpool.tile([128, S], f32r)   # rope(Q)^T rows 0-63 h0, 64-127 h1
            KTr = cpool.tile([128, S], f32r)   # rope(K)^T duplicated
            Vp = cpool.tile([128, S // 128, DH + 1], bf16)  # V[k, dh] + ones
            identb = cpool.tile([DH, DH], bf16)
            identh = cpool.tile([128, 128], f16)
            biasc = cpool.tile([128, 1], f32)

            nc.sync.dma_start(
                out=wq_sb, in_=wqT[:, :].rearrange("(c p) m -> p c m", p=128))
            nc.sync.dma_start(
                out=wk_sb, in_=wkTd[:, :].rearrange("(c p) m -> p c m", p=128))
            make_identity(nc, identb[:, :])
            make_identity(nc, identh[:, :])
            nc.vector.memset(biasc, float(EXP_BIAS))
            nc.vector.memset(Vp[:, :, DH:DH + 1], 1.0)

            xb_tiles = {}
            ON_t = {}
            ONT_t = {}
            SHUF = [i ^ 1 for i in range(32)]

            def emit_xb_dma(t, half):
                # split x by COLUMN GROUP (not d-chunk): a Q/K/V projection
                # stream for column group `half` then depends on only ONE
                # 5.8us transfer instead of two
                xb = xb_pool.tile([128, 8, 512], f32r, tag=f"xb{half}",
                                  name=f"xb_{t}_{half}")
                xb_tiles[(t, half)] = xb
                c0 = t * QT + half * 512
                src = xT[:, c0:c0 + 512]
                nc.sync.dma_start(out=xb,
                                  in_=src.rearrange("(c p) q -> p c q", p=128))

            def emit_cs_dma(t, sgh=None, eng=None):
                # sync queue: the ScalarE sequencer must stay DMA-free so it
                # can dispatch exps (DMA issue blocks the issuing SEQ on the
                # serialized HWDGE)
                eng = eng or nc.sync
                halves = (0, 1) if sgh is None else (sgh,)
                for hh in halves:
                    c0 = t * QT + hh * 512
                    sl = bass.ds(c0, 512)
                    eng.dma_start(out=cos_sb[:, sl], in_=cosT[:, c0:c0 + 512])
                    eng.dma_start(out=sin_sb[:, sl],
                                  in_=sinTs[:, c0:c0 + 512])

            def micro_qk(t, sgh, which, pool, tag):
                """Q/K projection stream + rope as a list of micro-ops."""
                sg = 2 * t + sgh
                st = {}
                w_sb = wq_sb if which == "q" else wk_sb
                dst = QTr if which == "q" else KTr
                scols = bass.ds(sg * 512, 512)

                def mk_mm(cd):
                    def f():
                        if "ps" not in st:
                            st["ps"] = pool.tile([128, 512], f32, tag=tag,
                                                 name=f"{which}t_{sg}")
                        nc.tensor.matmul(
                            st["ps"][:, :], w_sb[:, cd, :],
                            xb_tiles[(t, sgh)][:, cd, :],
                            start=(cd == 0), stop=(cd == 7))
                    return f

                def rope_a():
                    st["m1"] = rtmp.tile([128, 512], f32, tag="m1", name=f"m1_{which}_{sg}")
                    st["m2"] = rtmp.tile([128, 512], f32, tag="m2", name=f"m2_{which}_{sg}")
                    nc.vector.tensor_mul(st["m1"], st["ps"][:, :],
                                         cos_sb[:, scols])
                    nc.vector.tensor_mul(st["m2"], st["ps"][:, :],
                                         sin_sb[:, scols])

                def rope_b():
                    sh = rtmp.tile([128, 512], f32, tag="sh")
                    nc.vector.stream_shuffle(sh, st["m2"], SHUF)
                    nc.vector.tensor_add(dst[:, scols], st["m1"], sh)

                return [mk_mm(cd) for cd in range(8)] + [rope_a, rope_b]

            def micro_vt(t, sgh):
                """V projection + transpose into Vp, as micro-ops."""
                sg = 2 * t + sgh
                st = {}

                def mk_mm(cd):
                    def f():
                        if "ps" not in st:
                            st["ps"] = psB.tile([DH, 512], f32, tag="op",
                                                name=f"vt_{sg}")
                        nc.tensor.matmul(
                            st["ps"][:, :], wv_sb[:, cd, :],
                            xb_tiles[(t, sgh)][:, cd, :],
                            start=(cd == 0), stop=(cd == 7))
                    return f

                def cp():
                    st["vs"] = vt_pool.tile([DH, 512], bf16, tag="vt", name=f"vs_{sg}")
                    nc.vector.tensor_copy(st["vs"], st["ps"][:, :])

                def mk_tr(i):
                    def f():
                        kc = sg * 4 + i
                        tr = psB.tile([128, DH], bf16, tag="op",
                                      name=f"vtr_{kc}")
                        nc.tensor.transpose(
                            tr[:, :], st["vs"][:, i * 128:(i + 1) * 128],
                            identb[:, :])
                        nc.vector.tensor_copy(Vp[:, kc, 0:DH], tr[:, :])
                    return f

                return ([mk_mm(cd) for cd in range(8)] + [cp]
                        + [mk_tr(i) for i in range(4)])

            # Schraudolph-style integer exp producing bf16 directly:
            #   e = bitcast_bf16(uint16(max(A*s + B, 0)))   (~4% max rel err)
            # used to offload some exps from the saturated ScalarE onto
            # DVE (affine, PSUM read) + GpSimd (clamp + u16 convert).
            SCH_A = float(128.0 / np.log(2.0))
            SCH_B = float(127 * 128 - 4.0 + SCH_A * EXP_BIAS)
            u16 = mybir.dt.uint16

            def emit_scores_exp(t, h, c, seg=None, e=None, approx=False):
                """Scores + exp for key-chunk c over q-columns [lo, hi) of
                the tile (default: the full causal suffix)."""
                q0 = t * QT
                qs = max(0, (c - 8 * t) * 128)
                lo0, hi0 = (qs, QT) if seg is None else seg
                s_ps = psA.tile([128, QT], f32, tag=f"s{c % 2}",
                                name=f"s_{t}_{h}_{c}_{lo0}")
                lhs = KTr[64 * h:64 * h + 64, c * 128:(c + 1) * 128]
                for lo, hi in ((lo0, min(hi0, 512)), (max(lo0, 512), hi0)):
                    if lo >= hi:
                        continue
                    nc.tensor.matmul(
                        s_ps[:, bass.ds(lo, hi - lo)], lhs,
                        QTr[64 * h:64 * h + 64, q0 + lo:q0 + hi],
                        start=True, stop=True)
                if e is None:
                    e = e_pool.tile([128, QT], bf16, tag="e",
                                    name=f"e_{t}_{h}_{c}")
                if approx:
                    tx = tx_pool.tile([128, QT], f32, tag="tx",
                                      name=f"tx_{t}_{h}_{c}")
                    nc.vector.tensor_scalar(
                        tx[:, lo0:hi0], s_ps[:, lo0:hi0], SCH_A, SCH_B,
                        mybir.AluOpType.mult, mybir.AluOpType.add)
                    nc.gpsimd.tensor_scalar(
                        e[:, lo0:hi0].bitcast(u16), tx[:, lo0:hi0],
                        0.0, None, mybir.AluOpType.max)
                else:
                    nc.scalar.activation(
                        e[:, lo0:hi0], s_ps[:, lo0:hi0],
                        mybir.ActivationFunctionType.Exp,
                        bias=biasc[:, :], scale=1.0)
                if c >= 8 * t and lo0 <= qs < hi0:
                    nc.gpsimd.affine_select(
                        out=e[:, qs:qs + 128], in_=e[:, qs:qs + 128],
                        pattern=[[1, 128]],
                        compare_op=mybir.AluOpType.is_ge,
                        fill=0.0, base=0, channel_multiplier=-1)
                return e

            def emit_pv(t, c, e, bankA, bankB):
                for j in range(max(0, c - 8 * t), 8):
                    bank, jj = (bankA, j) if j < 4 else (bankB, j - 4)
                    last_c = 8 * t + (3 if j < 4 else 7)
                    nc.tensor.matmul(
                        bank[:, jj, :], e[:, j * 128:(j + 1) * 128],
                        Vp[:, c, :],
                        start=(c == 0 and jj == 0),
                        stop=(c == last_c and jj == 3))

            def emit_norm(t, h, bank, jbase, js):
                rc = rc_pool.tile([128, 4, 1], f32, tag="rc",
                                  name=f"rc_{t}_{h}_{jbase}_{js[0]}")
                j0, j1 = js[0], js[-1] + 1
                nc.vector.reciprocal_approx_fast(
                    rc[:, 0:j1 - j0, :], bank[:, j0:j1, DH:DH + 1])
                for j4 in js:
                    nc.vector.tensor_scalar_mul(
                        ON_t[t][:, jbase + j4, 64 * h:64 * h + 64],
                        bank[:, j4, 0:DH], rc[:, j4 - j0, :])

            def emit_ont(t, j, tail=False):
                tr = psB.tile([128, 128], f16, tag="op", name=f"ontr_{t}_{j}")
                nc.tensor.transpose(tr[:, :], ON_t[t][:, j, :], identh[:, :])
                if tail:  # ScalarE is idle during the kernel tail
                    nc.scalar.copy(ONT_t[t][:, j, :], tr[:, :])
                else:
                    nc.vector.tensor_copy(ONT_t[t][:, j, :], tr[:, :])

            def micro_oproj(t, j, split_eng=False):
                """Output projection for q-chunk j of tile t: two half-units.
                split_eng puts the first staging copy on ScalarE (tail mode,
                when ScalarE has gone idle)."""
                st = {}

                def half(dseg):
                    def f():
                        if "ob" not in st:
                            st["ob"] = ob_pool.tile([128, QT], f16, tag="ob", name=f"ob_{t}_{j}")
                        op = psB.tile([128, 512], f32, tag="op",
                                      name=f"op_{t}_{j}_{dseg}")
                        nc.tensor.matmul(
                            op[:, :], ONT_t[t][:, j, :],
                            wo_sb[:, dseg * 512:(dseg + 1) * 512],
                            start=True, stop=True)
                        eng = nc.scalar if (split_eng and dseg == 0) \
                            else nc.vector
                        if eng is nc.scalar:
                            eng.copy(st["ob"][:, dseg * 512:(dseg + 1) * 512],
                                     op[:, :])
                        else:
                            eng.tensor_copy(
                                st["ob"][:, dseg * 512:(dseg + 1) * 512],
                                op[:, :])
                        if dseg == 1:
                            nc.sync.dma_start(
                                out=out[t * QT + j * 128:
                                        t * QT + (j + 1) * 128, :],
                                in_=st["ob"])
                    return f

                return [half(0), half(1)]

            # ---- prologue: only what chunk 0-3's first 512 q-columns need
            # (Q/K of column group 0); the rest is emitted at the phase
            # boundary inside tile 0 so the first exps start ~15us earlier.
            # DMA issue order is deliberate: small weight/table DMAs first,
            # then the big x transfers, all on the sync queue.
            emit_cs_dma(0, sgh=0)
            emit_xb_dma(0, 0)
            emit_cs_dma(0, sgh=1)
            emit_xb_dma(0, 1)
            nc.sync.dma_start(
                out=wv_sb, in_=wvT[:, :].rearrange("(c p) m -> p c m", p=128))
            for f in micro_qk(0, 0, "q", psA, "s0"):
                f()
            for f in micro_qk(0, 0, "k", psA, "s1"):
                f()

            pend = {"pv": None, "tail": None}
            for t in range(NQT):
                NCH = 8 * (t + 1)
                last_t = t == NQT - 1
                ON_t[t] = on_pool.tile([128, 8, 128], f16, tag="ON",
                                       name=f"ON_{t}")
                if last_t:
                    ONT_t[t] = ont_pool.tile([128, 8, 128], f16, tag="ONT",
                                             name=f"ONT_{t}")
                # next tile's input DMAs first (latency-critical)
                if t + 1 < NQT:
                    emit_xb_dma(t + 1, 0)
                    emit_xb_dma(t + 1, 1)
                    emit_cs_dma(t + 1)
                if t == 0:
                    nc.sync.dma_start(out=wo_sb, in_=woT[:, :])

                # early queue: ONT transposes of t-1 + scheduled oproj units
                early = []
                if t >= 1:
                    ONT_t[t - 1] = ont_pool.tile([128, 8, 128], f16,
                                                 tag="ONT",
                                                 name=f"ONT_{t-1}")
                    for j in range(8):
                        early.append(lambda t=t, j=j: emit_ont(t - 1, j))
                # oproj schedule: t0 -> tile2, t1 and t2 -> tile3
                osrc = {2: [0], 3: [1, 2]}.get(t, [])
                for ot in osrc:
                    for j in range(8):
                        early.extend(micro_oproj(ot, j))
                # late queue: projections for tile t+1 (needs xb DMA landed)
                late = []
                if t + 1 < NQT:
                    order = ([(0, "q"), (0, "k"), (1, "q"), (1, "k")]
                             if t == 0 else
                             [(0, "q"), (0, "k"), (1, "q"), (1, "k")])
                    for sgh, which in order:
                        late.extend(micro_qk(t + 1, sgh, which, psB, "op"))
                    late.extend(micro_vt(t + 1, 0))
                    late.extend(micro_vt(t + 1, 1))

                iters = 2 * NCH
                n_early = len(early)
                n_late = len(late)
                done_iters = 0
                e_popped = l_popped = 0
                LATE_FRAC = 0.30 if t == 0 else 0.45
                for h in range(2):
                    es0 = {}
                    if t == 0 and h == 0:
                        # phase A: first 512 q-columns of chunks 0-3 need
                        # only column-group-0 Q/K (already roped) — start
                        # ScalarE while the rest of the projections build
                        for c in range(4):
                            es0[c] = emit_scores_exp(0, 0, c,
                                                     seg=(128 * c, 512))
                        # phase boundary: column-group-1 Q/K + V projections
                        for f in micro_qk(0, 1, "q", psB, "op"):
                            f()
                        for f in micro_qk(0, 1, "k", psB, "op"):
                            f()
                        for f in micro_vt(0, 0) + micro_vt(0, 1):
                            f()
                    bankA = psA.tile([128, 4, DH + 1], f32, tag="oaccA",
                                     name=f"oA_{t}_{h}")
                    bankB = psA.tile([128, 4, DH + 1], f32, tag="oaccB",
                                     name=f"oB_{t}_{h}")
                    for c in range(NCH):
                        off = False  # ScalarE->DVE/GpSimd exp offload: net loss (queue serialization)
                        if c in es0:
                            e = emit_scores_exp(t, h, c, seg=(512, QT),
                                                e=es0[c])
                        else:
                            e = emit_scores_exp(t, h, c, approx=off)
                        if pend["pv"] is not None:
                            emit_pv(*pend["pv"])
                            pend["pv"] = None
                        if pend["tail"] is not None:
                            pend["tail"]()
                            pend["tail"] = None
                        pend["pv"] = (t, c, e, bankA, bankB)
                        cj = c - 8 * t
                        if cj == 4:
                            emit_norm(t, h, bankA, 0, (0, 1, 2, 3))
                            if last_t and h == 1:
                                for j in range(4):
                                    emit_ont(t, j)
                                    for f in micro_oproj(t, j,
                                                         split_eng=True):
                                        f()
                        if last_t and cj >= 5:
                            jd = cj - 1
                            emit_norm(t, h, bankB, 4, (jd - 4,))
                            if h == 1:
                                emit_ont(t, jd)
                                for f in micro_oproj(t, jd, split_eng=True):
                                    f()
                        done_iters += 1
                        et = (n_early * done_iters * 4 + 3 * iters) \
                            // (3 * iters)
                        while e_popped < min(et, n_early):
                            early[e_popped]()
                            e_popped += 1
                        prog = done_iters / iters
                        if prog > LATE_FRAC:
                            lt = int(n_late * (prog - LATE_FRAC)
                                     / (0.95 - LATE_FRAC)) + 1
                            while l_popped < min(lt, n_late):
                                late[l_popped]()
                                l_popped += 1
                    # defer this head's final PV + bank-B norm past the next
                    # head's/tile's first scores+exp (no PE head-block)
                    if not (last_t and h == 1):
                        def _tail(t=t, h=h, bankB=bankB, pv=pend["pv"],
                                  lt=last_t):
                            emit_pv(*pv)
                            emit_norm(t, h, bankB, 4, (3,) if lt else
                                      (0, 1, 2, 3))
                        pend["pv"] = None
                        pend["tail"] = _tail
                    else:
                        emit_pv(*pend["pv"])
                        pend["pv"] = None
                        emit_norm(t, h, bankB, 4, (3,))
                        emit_ont(t, 7)
                        for f in micro_oproj(t, 7, split_eng=True):
                            f()
                while e_popped < n_early:
                    early[e_popped]()
                    e_popped += 1
                while l_popped < n_late:
                    late[l_popped]()
                    l_popped += 1

    nc.compile()
    return nc


def _host_inputs(x, wq, wk, wv, wo):
    """Build the 8 per-core input dicts."""
    x2 = np.ascontiguousarray(x.reshape(S, D))
    xT = np.ascontiguousarray(x2.T)

    # rope pair-interleaved dh order: [0, 32, 1, 33, ...]
    perm = np.empty(DH, dtype=np.int64)
    perm[0::2] = np.arange(DH // 2)
    perm[1::2] = np.arange(DH // 2) + DH // 2

    inv_freq = 1.0 / (ROPE_THETA ** (np.arange(0, DH, 2, dtype=np.float64) / DH))
    ang = np.arange(S, dtype=np.float64)[:, None] * inv_freq[None, :]  # [S, 32]
    cosv = np.cos(ang)
    sinv = np.sin(ang)
    C64 = np.empty((DH, S), dtype=np.float32)
    Ss64 = np.empty((DH, S), dtype=np.float32)
    for j in range(DH):
        C64[j] = cosv[:, j // 2]
        Ss64[j] = sinv[:, j // 2] * (1.0 if j % 2 == 0 else -1.0)
    cosT = np.ascontiguousarray(np.tile(C64, (2, 1)))
    sinTs = np.ascontiguousarray(np.tile(Ss64, (2, 1)))

    wq4 = wq.reshape(H, DH, D)
    wk4 = wk.reshape(HKV, DH, D)
    wv4 = wv.reshape(HKV, DH, D)

    ins = []
    for c in range(NCORES):
        h0, h1 = 2 * c, 2 * c + 1
        g = h0 // (H // HKV)
        wq_c = np.concatenate([wq4[h0][perm], wq4[h1][perm]], axis=0)  # [128, D]
        wk_c = np.concatenate([wk4[g][perm], wk4[g][perm]], axis=0)    # [128, D]
        wo_c = wo[:, np.r_[h0 * DH:(h0 + 1) * DH, h1 * DH:(h1 + 1) * DH]]
        ins.append({
            "xT": xT,
            "wqT": np.ascontiguousarray(wq_c.T),
            "wkTd": np.ascontiguousarray(wk_c.T),
            "wvT": np.ascontiguousarray(wv4[g].T),
            "woT": np.ascontiguousarray(wo_c.T).astype(np.float16),
            "cosT": cosT,
            "sinTs": sinTs,
        })
    return ins


def _is_causal(mask):
    if mask.shape != (S, S):
        return False
    expected = np.where(np.tril(np.ones((S, S), dtype=bool)), np.float32(0.0),
                        np.float32(-1e9))
    return np.array_equal(mask, expected)


def run_cores(x, mask, wq, wk, wv, wo, **spmd_kwargs):
    from concourse.bass_utils import run_bass_kernel_spmd

    causal = _is_causal(np.asarray(mask))
    assert causal, "v2 fast path requires the standard causal mask"
    if True not in _cache:
        _cache[True] = _build_fast()
    nc = _cache[True]

    ins = _host_inputs(np.asarray(x), np.asarray(wq), np.asarray(wk),
                       np.asarray(wv), np.asarray(wo))
    res = run_bass_kernel_spmd(nc, ins, core_ids=list(range(NCORES)),
                               **spmd_kwargs)
    return res


def _build(causal):
    assert causal
    return _build_fast()


def kernel(x, mask, wq, wk, wv, wo):
    res = run_cores(x, mask, wq, wk, wv, wo)
    acc = np.zeros((S, D), dtype=np.float64)
    for r in res.results:
        acc += r["out"].astype(np.float64)
    return acc.astype(np.float32).reshape(B, S, D)


# revision 7
# speedup vs baseline: 1.0379x; 1.0012x over previous
"""GQA causal attention (B=1, S=4096, D=1024, H=16, HKV=4, Dh=64, RoPE) on
8 Trainium2 NeuronCores — v2 (software-pipelined single fused pass).

Sharding: 8-way head parallelism as v1 (core c owns query heads {2c, 2c+1},
sharing KV head c//2; host sums the 8 partial output projections in f64).

Device program (4 q-tiles of 1024, one TileContext):
  - Per tile t: scores S^T[k, q] per 128-key chunk (fp32r, two 512-wide
    matmuls into a double-buffered [128,1024] PSUM pair), exp on ScalarE
    (PSUM in, bf16 out, fixed bias -10 — softmax-shift-invariant), diagonal
    triangle zeroed by gpsimd affine_select. exp is the metronome: ScalarE
    runs one 0.9-1us exp per (head, chunk) and everything else is scheduled
    around keeping it saturated.
  - PV flipped: out[q(128), dh+1] accumulated per (q-chunk, k-chunk), e
    stationary, V[k, dh|ones] bf16 as 65-row moving operand (65 rows/matmul
    vs q-width in the natural orientation). Ones column -> per-partition
    softmax denominator, so normalize is reciprocal + tensor_scalar. The 8
    q-chunk accumulators live in 2 PSUM banks as interleaved accumulation
    groups (single bank-clearing start, per-element pending-zero handles
    first-write-overwrite). PV is deferred one chunk so it never blocks the
    next chunk's scores in the in-order PE queue.
  - Normalized O[q, hd] (f16) is PE-transposed to O^T per q-chunk for the
    output projection (f16 weights), staged f16, DMA'd per q-chunk row.
  - Cross-tile software pipeline: projections for tile t+1 (Q/K via
    w-stationary streams + rope; V via dh-stationary stream + PE transpose)
    and the previous tiles' output projections are split into ~0.2-0.7us
    micro-ops drained between chunk emissions, so no insertion head-blocks
    the in-order engine queues. Output projections are scheduled into the
    LATER tiles (t0->t2, t1,t2->t3) where ScalarE is the local bottleneck
    and PE has slack. The last tile's tail is normed per-q-chunk the moment
    its accumulator completes so the output tail overlaps the final chunks.
PSUM: s0,s1 (2 banks each) + oaccA,oaccB (1+1) + 2 rotating "op" banks = 8.
"""

import os

import numpy as np

B, S, D = 1, 4096, 1024
H, HKV, DH = 16, 4, 64
NCORES = 8
ROPE_THETA = 10000.0
QT = 1024
NQT = S // QT
EXP_BIAS = -10.0

_cache = {}


def _build_fast():
    import concourse.bass as bass
    import concourse.tile as tile
    from concourse import bacc, mybir
    from concourse.masks import make_identity

    f32 = mybir.dt.float32
    f32r = mybir.dt.float32r
    bf16 = mybir.dt.bfloat16
    f16 = mybir.dt.float16

    nc = bacc.Bacc(None, target_bir_lowering=False)

    xT = nc.dram_tensor("xT", [D, S], f32r, kind="ExternalInput")
    wqT = nc.dram_tensor("wqT", [D, 128], f32r, kind="ExternalInput")
    wkTd = nc.dram_tensor("wkTd", [D, 128], f32r, kind="ExternalInput")
    wvT = nc.dram_tensor("wvT", [D, DH], f32r, kind="ExternalInput")
    woT = nc.dram_tensor("woT", [128, D], f16, kind="ExternalInput")
    cosT = nc.dram_tensor("cosT", [128, S], f32, kind="ExternalInput")
    sinTs = nc.dram_tensor("sinTs", [128, S], f32, kind="ExternalInput")
    out = nc.dram_tensor("out", [S, D], f16, kind="ExternalOutput")

    with tile.TileContext(nc) as tc:
        with tc.tile_pool(name="const", bufs=1) as cpool, \
             tc.tile_pool(name="xb", bufs=2) as xb_pool, \
             tc.tile_pool(name="rtmp", bufs=4) as rtmp, \
             tc.tile_pool(name="esb", bufs=7) as e_pool, \
             tc.tile_pool(name="onp", bufs=2) as on_pool, \
             tc.tile_pool(name="ontp", bufs=4) as ont_pool, \
             tc.tile_pool(name="vts", bufs=3) as vt_pool, \
             tc.tile_pool(name="rcp", bufs=2) as rc_pool, \
             tc.tile_pool(name="txp", bufs=2) as tx_pool, \
             tc.tile_pool(name="obp", bufs=4) as ob_pool, \
             tc.tile_pool(name="psA", bufs=1, space="PSUM") as psA, \
             tc.tile_pool(name="psB", bufs=2, space="PSUM") as psB:

            # ---- resident constants ----
            wq_sb = cpool.tile([128, 8, 128], f32r)
            wk_sb = cpool.tile([128, 8, 128], f32r)
            wv_sb = cpool.tile([128, 8, DH], f32r)
            wo_sb = cpool.tile([128, D], f16)
            cos_sb = cpool.tile([128, S], f32)
            sin_sb = cpool.tile([128, S], f32)
            QTr = cpool.tile([128, S], f32r)   # rope(Q)^T rows 0-63 h0, 64-127 h1
            KTr = cpool.tile([128, S], f32r)   # rope(K)^T duplicated
            Vp = cpool.tile([128, S // 128, DH + 1], bf16)  # V[k, dh] + ones
            identb = cpool.tile([DH, DH], bf16)
            identh = cpool.tile([128, 128], f16)
            biasc = cpool.tile([128, 1], f32)

            nc.sync.dma_start(
                out=wq_sb, in_=wqT[:, :].rearrange("(c p) m -> p c m", p=128))
            nc.sync.dma_start(
                out=wk_sb, in_=wkTd[:, :].rearrange("(c p) m -> p c m", p=128))
            make_identity(nc, identb[:, :])
            make_identity(nc, identh[:, :])
            nc.vector.memset(biasc, float(EXP_BIAS))
            nc.vector.memset(Vp[:, :, DH:DH + 1], 1.0)

            xb_tiles = {}
            ON_t = {}
            ONT_t = {}
            SHUF = [i ^ 1 for i in range(32)]

            def emit_xb_dma(t, half):
                # split x by COLUMN GROUP (not d-chunk): a Q/K/V projection
                # stream for column group `half` then depends on only ONE
                # 5.8us transfer instead of two
                xb = xb_pool.tile([128, 8, 512], f32r, tag=f"xb{half}",
                                  name=f"xb_{t}_{half}")
                xb_tiles[(t, half)] = xb
                c0 = t * QT + half * 512
                src = xT[:, c0:c0 + 512]
                nc.sync.dma_start(out=xb,
                                  in_=src.rearrange("(c p) q -> p c q", p=128))

            def emit_cs_dma(t, sgh=None, eng=None):
                # sync queue: the ScalarE sequencer must stay DMA-free so it
                # can dispatch exps (DMA issue blocks the issuing SEQ on the
                # serialized HWDGE)
                eng = eng or nc.sync
                halves = (0, 1) if sgh is None else (sgh,)
                for hh in halves:
                    c0 = t * QT + hh * 512
                    sl = bass.ds(c0, 512)
                    eng.dma_start(out=cos_sb[:, sl], in_=cosT[:, c0:c0 + 512])
                    eng.dma_start(out=sin_sb[:, sl],
                                  in_=sinTs[:, c0:c0 + 512])

            def micro_qk(t, sgh, which, pool, tag):
                """Q/K projection stream + rope as a list of micro-ops."""
                sg = 2 * t + sgh
                st = {}
                w_sb = wq_sb if which == "q" else wk_sb
                dst = QTr if which == "q" else KTr
                scols = bass.ds(sg * 512, 512)

                def mk_mm(cd):
                    def f():
                        if "ps" not in st:
                            st["ps"] = pool.tile([128, 512], f32, tag=tag,
                                                 name=f"{which}t_{sg}")
                        nc.tensor.matmul(
                            st["ps"][:, :], w_sb[:, cd, :],
                            xb_tiles[(t, sgh)][:, cd, :],
                            start=(cd == 0), stop=(cd == 7))
                    return f

                def rope_a():
                    st["m1"] = rtmp.tile([128, 512], f32, tag="m1", name=f"m1_{which}_{sg}")
                    st["m2"] = rtmp.tile([128, 512], f32, tag="m2", name=f"m2_{which}_{sg}")
                    nc.vector.tensor_mul(st["m1"], st["ps"][:, :],
                                         cos_sb[:, scols])
                    nc.vector.tensor_mul(st["m2"], st["ps"][:, :],
                                         sin_sb[:, scols])

                def rope_b():
                    sh = rtmp.tile([128, 512], f32, tag="sh")
                    nc.vector.stream_shuffle(sh, st["m2"], SHUF)
                    nc.vector.tensor_add(dst[:, scols], st["m1"], sh)

                return [mk_mm(cd) for cd in range(8)] + [rope_a, rope_b]

            def micro_vt(t, sgh):
                """V projection + transpose into Vp, as micro-ops."""
                sg = 2 * t + sgh
                st = {}

                def mk_mm(cd):
                    def f():
                        if "ps" not in st:
                            st["ps"] = psB.tile([DH, 512], f32, tag="op",
                                                name=f"vt_{sg}")
                        nc.tensor.matmul(
                            st["ps"][:, :], wv_sb[:, cd, :],
                            xb_tiles[(t, sgh)][:, cd, :],
                            start=(cd == 0), stop=(cd == 7))
                    return f

                def cp():
                    st["vs"] = vt_pool.tile([DH, 512], bf16, tag="vt", name=f"vs_{sg}")
                    nc.vector.tensor_copy(st["vs"], st["ps"][:, :])

                def mk_tr(i):
                    def f():
                        kc = sg * 4 + i
                        tr = psB.tile([128, DH], bf16, tag="op",
                                      name=f"vtr_{kc}")
                        nc.tensor.transpose(
                            tr[:, :], st["vs"][:, i * 128:(i + 1) * 128],
                            identb[:, :])
                        nc.vector.tensor_copy(Vp[:, kc, 0:DH], tr[:, :])
                    return f

                return ([mk_mm(cd) for cd in range(8)] + [cp]
                        + [mk_tr(i) for i in range(4)])

            # Schraudolph-style integer exp producing bf16 directly:
            #   e = bitcast_bf16(uint16(max(A*s + B, 0)))   (~4% max rel err)
            # used to offload some exps from the saturated ScalarE onto
            # DVE (affine, PSUM read) + GpSimd (clamp + u16 convert).
            SCH_A = float(128.0 / np.log(2.0))
            SCH_B = float(127 * 128 - 4.0 + SCH_A * EXP_BIAS)
            u16 = mybir.dt.uint16

            def emit_scores_exp(t, h, c, seg=None, e=None, approx=False):
                """Scores + exp for key-chunk c over q-columns [lo, hi) of
                the tile (default: the full causal suffix)."""
                q0 = t * QT
                qs = max(0, (c - 8 * t) * 128)
                lo0, hi0 = (qs, QT) if seg is None else seg
                s_ps = psA.tile([128, QT], f32, tag=f"s{c % 2}",
                                name=f"s_{t}_{h}_{c}_{lo0}")
                lhs = KTr[64 * h:64 * h + 64, c * 128:(c + 1) * 128]
                for lo, hi in ((lo0, min(hi0, 512)), (max(lo0, 512), hi0)):
                    if lo >= hi:
                        continue
                    nc.tensor.matmul(
                        s_ps[:, bass.ds(lo, hi - lo)], lhs,
                        QTr[64 * h:64 * h + 64, q0 + lo:q0 + hi],
                        start=True, stop=True)
                if e is None:
                    e = e_pool.tile([128, QT], bf16, tag="e",
                                    name=f"e_{t}_{h}_{c}")
                if approx:
                    tx = tx_pool.tile([128, QT], f32, tag="tx",
                                      name=f"tx_{t}_{h}_{c}")
                    nc.vector.tensor_scalar(
                        tx[:, lo0:hi0], s_ps[:, lo0:hi0], SCH_A, SCH_B,
                        mybir.AluOpType.mult, mybir.AluOpType.add)
                    nc.gpsimd.tensor_scalar(
                        e[:, lo0:hi0].bitcast(u16), tx[:, lo0:hi0],
                        0.0, None, mybir.AluOpType.max)
                else:
                    nc.scalar.activation(
                        e[:, lo0:hi0], s_ps[:, lo0:hi0],
                        mybir.ActivationFunctionType.Exp,
                        bias=biasc[:, :], scale=1.0)
                if c >= 8 * t and lo0 <= qs < hi0:
                    nc.gpsimd.affine_select(
                        out=e[:, qs:qs + 128], in_=e[:, qs:qs + 128],
                        pattern=[[1, 128]],
                        compare_op=mybir.AluOpType.is_ge,
                        fill=0.0, base=0, channel_multiplier=-1)
                return e

            def emit_pv(t, c, e, bankA, bankB):
                for j in range(max(0, c - 8 * t), 8):
                    bank, jj = (bankA, j) if j < 4 else (bankB, j - 4)
                    last_c = 8 * t + (3 if j < 4 else 7)
                    nc.tensor.matmul(
                        bank[:, jj, :], e[:, j * 128:(j + 1) * 128],
                        Vp[:, c, :],
                        start=(c == 0 and jj == 0),
                        stop=(c == last_c and jj == 3))

            def emit_norm(t, h, bank, jbase, js):
                rc = rc_pool.tile([128, 4, 1], f32, tag="rc",
                                  name=f"rc_{t}_{h}_{jbase}_{js[0]}")
                j0, j1 = js[0], js[-1] + 1
                nc.vector.reciprocal_approx_fast(
                    rc[:, 0:j1 - j0, :], bank[:, j0:j1, DH:DH + 1])
                for j4 in js:
                    nc.vector.tensor_scalar_mul(
                        ON_t[t][:, jbase + j4, 64 * h:64 * h + 64],
                        bank[:, j4, 0:DH], rc[:, j4 - j0, :])

            def emit_ont(t, j, tail=False):
                tr = psB.tile([128, 128], f16, tag="op", name=f"ontr_{t}_{j}")
                nc.tensor.transpose(tr[:, :], ON_t[t][:, j, :], identh[:, :])
                if tail:  # ScalarE is idle during the kernel tail
                    nc.scalar.copy(ONT_t[t][:, j, :], tr[:, :])
                else:
                    nc.vector.tensor_copy(ONT_t[t][:, j, :], tr[:, :])

            def micro_oproj(t, j, split_eng=False):
                """Output projection for q-chunk j of tile t: two half-units.
                split_eng puts the first staging copy on ScalarE (tail mode,
                when ScalarE has gone idle)."""
                st = {}

                def half(dseg):
                    def f():
                        if "ob" not in st:
                            st["ob"] = ob_pool.tile([128, QT], f16, tag="ob", name=f"ob_{t}_{j}")
                        op = psB.tile([128, 512], f32, tag="op",
                                      name=f"op_{t}_{j}_{dseg}")
                        nc.tensor.matmul(
                            op[:, :], ONT_t[t][:, j, :],
                            wo_sb[:, dseg * 512:(dseg + 1) * 512],
                            start=True, stop=True)
                        eng = nc.scalar if (split_eng and dseg == 0) \
                            else nc.vector
                        if eng is nc.scalar:
                            eng.copy(st["ob"][:, dseg * 512:(dseg + 1) * 512],
                                     op[:, :])
                        else:
                            eng.tensor_copy(
                                st["ob"][:, dseg * 512:(dseg + 1) * 512],
                                op[:, :])
                        if dseg == 1:
                            nc.sync.dma_start(
                                out=out[t * QT + j * 128:
                                        t * QT + (j + 1) * 128, :],
                                in_=st["ob"])
                    return f

                return [half(0), half(1)]

            # ---- prologue: only what chunk 0-3's first 512 q-columns need
            # (Q/K of column group 0); the rest is emitted at the phase
            # boundary inside tile 0 so the first exps start ~15us earlier.
            # DMA issue order is deliberate: small weight/table DMAs first,
            # then the big x transfers, all on the sync queue.
            emit_cs_dma(0, sgh=0)
            emit_xb_dma(0, 0)
            emit_cs_dma(0, sgh=1)
            emit_xb_dma(0, 1)
            nc.sync.dma_start(
                out=wv_sb, in_=wvT[:, :].rearrange("(c p) m -> p c m", p=128))
            for f in micro_qk(0, 0, "q", psA, "s0"):
                f()
            for f in micro_qk(0, 0, "k", psA, "s1"):
                f()

            pend = {"pv": None, "tail": None}
            for t in range(NQT):
                NCH = 8 * (t + 1)
                last_t = t == NQT - 1
                ON_t[t] = on_pool.tile([128, 8, 128], f16, tag="ON",
                                       name=f"ON_{t}")
                if last_t:
                    ONT_t[t] = ont_pool.tile([128, 8, 128], f16, tag="ONT",
                                             name=f"ONT_{t}")
                # next tile's input DMAs first (latency-critical)
                if t + 1 < NQT:
                    emit_xb_dma(t + 1, 0)
                    emit_xb_dma(t + 1, 1)
                    emit_cs_dma(t + 1)
                if t == 0:
                    nc.sync.dma_start(out=wo_sb, in_=woT[:, :])

                # early queue: ONT transposes of t-1 + scheduled oproj units
                early = []
                if t >= 1:
                    ONT_t[t - 1] = ont_pool.tile([128, 8, 128], f16,
                                                 tag="ONT",
                                                 name=f"ONT_{t-1}")
                    for j in range(8):
                        early.append(lambda t=t, j=j: emit_ont(t - 1, j))
                # oproj schedule: t0 -> tile2, t1 and t2 -> tile3
                osrc = {3: [0, 1, 2]}.get(t, [])
                for ot in osrc:
                    for j in range(8):
                        early.extend(micro_oproj(ot, j))
                # late queue: projections for tile t+1 (needs xb DMA landed)
                late = []
                if t + 1 < NQT:
                    order = ([(0, "q"), (0, "k"), (1, "q"), (1, "k")]
                             if t == 0 else
                             [(0, "q"), (0, "k"), (1, "q"), (1, "k")])
                    for sgh, which in order:
                        late.extend(micro_qk(t + 1, sgh, which, psB, "op"))
                    late.extend(micro_vt(t + 1, 0))
                    late.extend(micro_vt(t + 1, 1))

                iters = 2 * NCH
                n_early = len(early)
                n_late = len(late)
                done_iters = 0
                e_popped = l_popped = 0
                LATE_FRAC = 0.30 if t == 0 else 0.45
                for h in range(2):
                    es0 = {}
                    if t == 0 and h == 0:
                        # phase A: first 512 q-columns of chunks 0-3 need
                        # only column-group-0 Q/K (already roped) — start
                        # ScalarE while the rest of the projections build
                        for c in range(4):
                            es0[c] = emit_scores_exp(0, 0, c,
                                                     seg=(128 * c, 512))
                        # phase boundary: column-group-1 Q/K + V projections
                        for f in micro_qk(0, 1, "q", psB, "op"):
                            f()
                        for f in micro_qk(0, 1, "k", psB, "op"):
                            f()
                        for f in micro_vt(0, 0) + micro_vt(0, 1):
                            f()
                    bankA = psA.tile([128, 4, DH + 1], f32, tag="oaccA",
                                     name=f"oA_{t}_{h}")
                    bankB = psA.tile([128, 4, DH + 1], f32, tag="oaccB",
                                     name=f"oB_{t}_{h}")
                    for c in range(NCH):
                        off = False  # ScalarE->DVE/GpSimd exp offload: net loss (queue serialization)
                        if c in es0:
                            e = emit_scores_exp(t, h, c, seg=(512, QT),
                                                e=es0[c])
                        else:
                            e = emit_scores_exp(t, h, c, approx=off)
                        if pend["pv"] is not None:
                            emit_pv(*pend["pv"])
                            pend["pv"] = None
                        if pend["tail"] is not None:
                            pend["tail"]()
                            pend["tail"] = None
                        pend["pv"] = (t, c, e, bankA, bankB)
                        cj = c - 8 * t
                        if cj == 4:
                            emit_norm(t, h, bankA, 0, (0, 1, 2, 3))
                            if last_t and h == 1:
                                for j in range(4):
                                    emit_ont(t, j)
                                    for f in micro_oproj(t, j,
                                                         split_eng=True):
                                        f()
                        if last_t and cj >= 5:
                            jd = cj - 1
                            emit_norm(t, h, bankB, 4, (jd - 4,))
                            if h == 1:
                                emit_ont(t, jd)
                                for f in micro_oproj(t, jd, split_eng=True):
                                    f()
                        done_iters += 1
                        et = (n_early * done_iters * 4 + 3 * iters) \
                            // (3 * iters)
                        while e_popped < min(et, n_early):
                            early[e_popped]()
                            e_popped += 1
                        prog = done_iters / iters
                        if prog > LATE_FRAC:
                            lt = int(n_late * (prog - LATE_FRAC)
                                     / (0.95 - LATE_FRAC)) + 1
                            while l_popped < min(lt, n_late):
                                late[l_popped]()
                                l_popped += 1
                    # defer this head's final PV + bank-B norm past the next
                    # head's/tile's first scores+exp (no PE head-block)
                    if not (last_t and h == 1):
                        def _tail(t=t, h=h, bankB=bankB, pv=pend["pv"],
                                  lt=last_t):
                            emit_pv(*pv)
                            emit_norm(t, h, bankB, 4, (3,) if lt else
                                      (0, 1, 2, 3))
                        pend["pv"] = None
                        pend["tail"] = _tail
                    else:
                        emit_pv(*pend["pv"])
                        pend["pv"] = None
                        emit_norm(t, h, bankB, 4, (3,))
                        emit_ont(t, 7)
                        for f in micro_oproj(t, 7, split_eng=True):
                            f()
                while e_popped < n_early:
                    early[e_popped]()
                    e_popped += 1
                while l_popped < n_late:
                    late[l_popped]()
                    l_popped += 1

    nc.compile()
    return nc


def _host_inputs(x, wq, wk, wv, wo):
    """Build the 8 per-core input dicts."""
    x2 = np.ascontiguousarray(x.reshape(S, D))
    xT = np.ascontiguousarray(x2.T)

    # rope pair-interleaved dh order: [0, 32, 1, 33, ...]
    perm = np.empty(DH, dtype=np.int64)
    perm[0::2] = np.arange(DH // 2)
    perm[1::2] = np.arange(DH // 2) + DH // 2

    inv_freq = 1.0 / (ROPE_THETA ** (np.arange(0, DH, 2, dtype=np.float64) / DH))
    ang = np.arange(S, dtype=np.float64)[:, None] * inv_freq[None, :]  # [S, 32]
    cosv = np.cos(ang)
    sinv = np.sin(ang)
    C64 = np.empty((DH, S), dtype=np.float32)
    Ss64 = np.empty((DH, S), dtype=np.float32)
    for j in range(DH):
        C64[j] = cosv[:, j // 2]
        Ss64[j] = sinv[:, j // 2] * (1.0 if j % 2 == 0 else -1.0)
    cosT = np.ascontiguousarray(np.tile(C64, (2, 1)))
    sinTs = np.ascontiguousarray(np.tile(Ss64, (2, 1)))

    wq4 = wq.reshape(H, DH, D)
    wk4 = wk.reshape(HKV, DH, D)
    wv4 = wv.reshape(HKV, DH, D)

    ins = []
    for c in range(NCORES):
        h0, h1 = 2 * c, 2 * c + 1
        g = h0 // (H // HKV)
        wq_c = np.concatenate([wq4[h0][perm], wq4[h1][perm]], axis=0)  # [128, D]
        wk_c = np.concatenate([wk4[g][perm], wk4[g][perm]], axis=0)    # [128, D]
        wo_c = wo[:, np.r_[h0 * DH:(h0 + 1) * DH, h1 * DH:(h1 + 1) * DH]]
        ins.append({
            "xT": xT,
            "wqT": np.ascontiguousarray(wq_c.T),
            "wkTd": np.ascontiguousarray(wk_c.T),
            "wvT": np.ascontiguousarray(wv4[g].T),
            "woT": np.ascontiguousarray(wo_c.T).astype(np.float16),
            "cosT": cosT,
            "sinTs": sinTs,
        })
    return ins


def _is_causal(mask):
    if mask.shape != (S, S):
        return False
    expected = np.where(np.tril(np.ones((S, S), dtype=bool)), np.float32(0.0),
                        np.float32(-1e9))
    return np.array_equal(mask, expected)


def run_cores(x, mask, wq, wk, wv, wo, **spmd_kwargs):
    from concourse.bass_utils import run_bass_kernel_spmd

    causal = _is_causal(np.asarray(mask))
    assert causal, "v2 fast path requires the standard causal mask"
    if True not in _cache:
        _cache[True] = _build_fast()
    nc = _cache[True]

    ins = _host_inputs(np.asarray(x), np.asarray(wq), np.asarray(wk),
                       np.asarray(wv), np.asarray(wo))
    res = run_bass_kernel_spmd(nc, ins, core_ids=list(range(NCORES)),
                               **spmd_kwargs)
    return res


def _build(causal):
    assert causal
    return _build_fast()


def kernel(x, mask, wq, wk, wv, wo):
    res = run_cores(x, mask, wq, wk, wv, wo)
    acc = np.zeros((S, D), dtype=np.float64)
    for r in res.results:
        acc += r["out"].astype(np.float64)
    return acc.astype(np.float32).reshape(B, S, D)
